# revision 1
# baseline (speedup 1.0000x reference)
"""Trainium2 Bass kernel for nn_ErrorAwareSelfAttention (8 NeuronCores).

Design (see inline notes):
- Stage A sharded by 8-image-row strips on cores 0-5 (window-aligned):
  k^T (ch-major) + v (pixel-major) projections, Modulator convs on frames
  0,1 (1-px halo from host-padded x_strip), pooled tokens sharded over all
  8 cores. One AllGather publishes k^T / v / pk^T / pv to every core.
- Stage B sharded 5-padded-windows per core: global attention per
  (window, head) with keys = 16x16 halo patch (the rolled+valid_ind key
  set is exactly that patch; attention is permutation-invariant over keys)
  + 576 pooled keys; local per-frame attention over the own 8x8 window;
  blend by the per-window mask flag; output projection; host scatters.
- Scores stay transposed (keys on partitions, 256 queries on free) so the
  softmax exp is a single ACT pass PSUM->SBUF and AV consumes p^T
  directly. Softmax denominators via PE-transpose + DVE row reduce; no
  max-subtraction (scores are O(1) by construction: 0.02-scale weights).
- All matmuls fp32 (4 cyc/row; float32r is broken on this platform: its
  DMA rounding contaminates unrelated transfers).
"""

import math
import sys

sys.path.insert(0, "/opt/trn_rl_repo")

import numpy as np

import concourse.bass as bass
import concourse.mybir as mybir
import concourse.tile as tile
from concourse import bacc
from concourse.bass_utils import run_bass_kernel_spmd
from concourse.masks import make_identity

dt = mybir.dt
AF = mybir.ActivationFunctionType
AX = mybir.AxisListType

# ---------------- problem constants (hardcoded) ----------------
DIM = 512
N_HEAD = 4
CH = 128
WH = WW = 8
EH = EW = 4
PH = PW = 4
B, T, HI, WI = 1, 4, 48, 48
L_T = 2
N_WH = N_WW = 6
NW = 36
WN = 64
SCALE = 1.0 / math.sqrt(CH)
N_CORES = 8
NC4 = 4  # 512 / 128 channel chunks

# stage A strips: 6 strips of 8 image rows (cores 0-5). x_strip has 1-row halo.
STRIP_H = 8
N_STRIP = 6
# stage B: 5 windows per core (padded; 36 windows total)
WPC = 5
_bounds = [int(NW * c / N_CORES) for c in range(N_CORES + 1)]
WIN_ASSIGN = []  # per core: list of 5 window ids (last repeated as padding)
for c in range(N_CORES):
    ws = list(range(_bounds[c], _bounds[c + 1]))
    while len(ws) < WPC:
        ws.append(ws[-1])
    WIN_ASSIGN.append(ws)

# pooled tokens: 12x12 per frame -> 576 rows, 72 per core
PGRID = HI // PH  # 12
NPOOL = T * PGRID * PGRID  # 576
POOL_PC = NPOOL // N_CORES  # 72

NPIX = T * HI * WI  # 9216
PATCH_PIX = 4 * 16 * 16  # 1024 keys/window from the halo patch (4 frames)
NKEYS = PATCH_PIX + NPOOL  # 1600 global keys
NQ = T * WN  # 256 queries per window
KCH = [128] * 8 + [POOL_PC] * 8  # 16 key chunks: 8 patch + 8 pooled(72)
NCHUNK = len(KCH)
# flat AllGather packing offsets (f32 elements)
OFF_K = 0
OFF_V = OFF_K + DIM * T * STRIP_H * 64
OFF_PK = OFF_V + T * STRIP_H * 64 * DIM
OFF_PV = OFF_PK + DIM * POOL_PC
AG_TOT = OFF_PV + POOL_PC * DIM

_NC_CACHE = {}


def _meta_for_core(c):
    """Per-window dynamic DMA registers: slot_prev, slot_cur, slot_next,
    x0 (patch x start in the 64-wide margin layout), x_own (=8j)."""
    vals = []
    for w in WIN_ASSIGN[c]:
        i, j = w // N_WW, w % N_WW
        vals += [(i - 1) % N_STRIP, i, (i + 1) % N_STRIP, (8 * j - 4) % 48, 8 * j]
    vals += [0] * (32 - len(vals) % 32 if len(vals) % 32 else 0)
    return np.asarray(vals[: ((len(vals) + 31) // 32) * 32], np.int32)


META_LEN = len(_meta_for_core(0))


def build_nc(debug=False):
    nc = bacc.Bacc("TRN2", target_bir_lowering=False, debug=True)

    # ---------------- I/O ----------------
    x_strip = nc.dram_tensor("x_strip", [T, STRIP_H + 2, WI, DIM], dt.float32,
                             kind="ExternalInput")
    x_win = nc.dram_tensor("x_win", [WPC, T, WN, DIM], dt.float32,
                           kind="ExternalInput")
    x_pool = nc.dram_tensor("x_pool", [POOL_PC // PGRID * PH, WI, DIM],
                            dt.float32, kind="ExternalInput")  # [24,48,512]
    mask_strip = nc.dram_tensor("mask_strip", [L_T, STRIP_H + 2, WI],
                                dt.float32, kind="ExternalInput")
    mask_win = nc.dram_tensor("mask_win", [L_T, WPC, WN], dt.float32,
                              kind="ExternalInput")
    halo_scale = nc.dram_tensor("halo_scale", [2], dt.float32,
                                kind="ExternalInput")
    meta = nc.dram_tensor("meta", [META_LEN], dt.int32, kind="ExternalInput")
    pool_ind = nc.dram_tensor("pool_ind", [2 * WI, PGRID], dt.float32,
                              kind="ExternalInput")  # [96,12] avg indicator

    wq_d = nc.dram_tensor("wq", [DIM, DIM], dt.float32, kind="ExternalInput")
    wk_d = nc.dram_tensor("wk", [DIM, DIM], dt.float32, kind="ExternalInput")
    wv_d = nc.dram_tensor("wv", [DIM, DIM], dt.float32, kind="ExternalInput")
    wp_d = nc.dram_tensor("wp", [DIM, DIM], dt.float32, kind="ExternalInput")
    bq_d = nc.dram_tensor("bq", [DIM], dt.float32, kind="ExternalInput")
    bk_d = nc.dram_tensor("bk", [DIM], dt.float32, kind="ExternalInput")
    bv_d = nc.dram_tensor("bv", [DIM], dt.float32, kind="ExternalInput")
    bp_d = nc.dram_tensor("bp", [DIM], dt.float32, kind="ExternalInput")
    pool_b_d = nc.dram_tensor("pool_b", [DIM], dt.float32, kind="ExternalInput")
    # modulator weights, host pre-transposed to [in,out]
    mods = {}
    for tag in ("k", "v"):
        mods[tag] = dict(
            sq=nc.dram_tensor(f"{tag}sq", [DIM, 128], dt.float32, kind="ExternalInput"),
            sqb=nc.dram_tensor(f"{tag}sqb", [128], dt.float32, kind="ExternalInput"),
            f=nc.dram_tensor(f"{tag}f", [9, 128, 128], dt.float32, kind="ExternalInput"),
            fb=nc.dram_tensor(f"{tag}fb", [128], dt.float32, kind="ExternalInput"),
            un=nc.dram_tensor(f"{tag}un", [128, DIM], dt.float32, kind="ExternalInput"),
            unb=nc.dram_tensor(f"{tag}unb", [DIM], dt.float32, kind="ExternalInput"),
        )

    out_win = nc.dram_tensor("out_win", [WPC, T, WN, DIM], dt.float32,
                             kind="ExternalOutput")
    dbg = {}
    if debug:
        dbg["k_contrib"] = nc.dram_tensor("dbg_k", [DIM, T, STRIP_H, 64],
                                          dt.float32, kind="ExternalOutput")
        dbg["v_contrib"] = nc.dram_tensor("dbg_v", [T, STRIP_H, 64, DIM],
                                          dt.float32, kind="ExternalOutput")
        dbg["pk"] = nc.dram_tensor("dbg_pk", [DIM, POOL_PC], dt.float32,
                                   kind="ExternalOutput")
        dbg["pv"] = nc.dram_tensor("dbg_pv", [POOL_PC, DIM], dt.float32,
                                   kind="ExternalOutput")
        dbg["q"] = nc.dram_tensor("dbg_q", [NC4, 128, WPC * NQ], dt.float32,
                                  kind="ExternalOutput")
        dbg["flags"] = nc.dram_tensor("dbg_flags", [WPC], dt.float32,
                                      kind="ExternalOutput")

    # internal DRAM for collective (single flat packed buffer)
    contrib = nc.dram_tensor("contrib", [1, AG_TOT], dt.float32)
    agout = nc.dram_tensor("agout", [N_CORES, AG_TOT], dt.float32,
                           addr_space="Shared")
    flags_d = nc.dram_tensor("flags_d", [WPC], dt.float32)

    with tile.TileContext(nc, num_cores=N_CORES) as tc:
        _program(nc, tc, locals())
    nc.compile()
    return nc


def _program(nc, tc, g):
    with (
        tc.tile_pool(name="consts", bufs=1) as consts,
        tc.tile_pool(name="wpool", bufs=1) as wpool,
        tc.tile_pool(name="ps_big", bufs=2, space="PSUM") as ps_big,
        tc.tile_pool(name="ps_mid", bufs=4, space="PSUM") as ps_mid,
        tc.tile_pool(name="ps_y", bufs=2, space="PSUM") as ps_y,
    ):
        ident = consts.tile([128, 128], dt.float32)
        make_identity(nc, ident)

        # weights as [128 (in chunk), 4 in-chunks, 512 out]
        W = {}
        for nm in ("wq", "wk", "wv", "wp"):
            t = wpool.tile([128, NC4, DIM], dt.float32, tag=nm)
            nc.sync.dma_start(t, g[nm + "_d"][:, :].rearrange("(a p) o -> p a o", p=128))
            W[nm] = t
        # per-partition bias tiles [128,1] x4 chunks: store as [128, 4]
        Bp = {}
        for nm in ("bq", "bk", "bv"):
            t = wpool.tile([128, NC4], dt.float32, tag=nm + "p")
            nc.sync.dma_start(t, g[nm + "_d"][:].rearrange("(a p) -> p a", p=128))
            Bp[nm] = t
        # free-axis broadcast bias tiles [128, 512]
        Bf = {}
        for nm in ("bv", "bp", "pool_b"):
            t = wpool.tile([128, DIM], dt.float32, tag=nm + "f")
            nc.sync.dma_start(t, g[nm + "_d"][:].unsqueeze(0).to_broadcast([128, DIM]))
            Bf[nm] = t
        MW = {}
        for tag in ("k", "v"):
            m = g["mods"][tag]
            MW[tag] = dict(
                sq=wpool.tile([128, NC4, 128], dt.float32, tag=f"{tag}sqw", name=f"{tag}sqw"),
                sqb=wpool.tile([128, 1], dt.float32, tag=f"{tag}sqbw", name=f"{tag}sqbw"),
                f=wpool.tile([128, 9, 128], dt.float32, tag=f"{tag}fw", name=f"{tag}fw"),
                fb=wpool.tile([128, 1], dt.float32, tag=f"{tag}fbw", name=f"{tag}fbw"),
                un=wpool.tile([128, DIM], dt.float32, tag=f"{tag}unw", name=f"{tag}unw"),
                unb=wpool.tile([128, NC4], dt.float32, tag=f"{tag}unbw", name=f"{tag}unbw"),
            )
            nc.sync.dma_start(MW[tag]["sq"], m["sq"][:, :].rearrange("(a p) o -> p a o", p=128))
            nc.sync.dma_start(MW[tag]["sqb"], m["sqb"][:].unsqueeze(1))
            nc.sync.dma_start(MW[tag]["f"], m["f"][:, :, :].rearrange("n p o -> p n o"))
            nc.sync.dma_start(MW[tag]["fb"], m["fb"][:].unsqueeze(1))
            nc.sync.dma_start(MW[tag]["un"], m["un"][:, :])
            nc.sync.dma_start(MW[tag]["unb"], m["unb"][:].rearrange("(a p) -> p a", p=128))
        pool_ind_t = wpool.tile([2 * WI, PGRID], dt.float32)
        nc.sync.dma_start(pool_ind_t, g["pool_ind"][:, :])
        hs = wpool.tile([128, 2], dt.float32)
        nc.sync.dma_start(hs, g["halo_scale"][:].unsqueeze(0).to_broadcast([128, 2]))

        # ---------------- stage B prologue: x_win^T, q^T (no AG dep) -------
        with tc.tile_pool(name="sbB0", bufs=1) as sbB0:
            # ================= stage A =================
            with tc.tile_pool(name="sbA", bufs=1) as sbA:
                _stage_a(nc, tc, g, sbA, ps_big, ps_mid, W, Bp, Bf, MW,
                         pool_ind_t, hs, ident)

            qT = sbB0.tile([128, NC4, WPC * NQ], dt.float32)
            with tc.tile_pool(name="sbXW", bufs=1) as sbXW:
                xtw = sbXW.tile([128, NC4, WPC * NQ], dt.float32)
                _transpose_in(nc, ps_big, sbXW,
                              g["x_win"][:, :, :, :].rearrange("w t p c -> (w t p) c"),
                              xtw, WPC * NQ, ident)
                for oc in range(NC4):
                    for piece in range(4):
                        s = piece * 320
                        ps = ps_big.tile([128, 320], dt.float32, tag="big")
                        for ic in range(NC4):
                            nc.tensor.matmul(ps, W["wq"][:, ic, oc * 128:(oc + 1) * 128],
                                             xtw[:, ic, s:s + 320],
                                             start=(ic == 0), stop=(ic == NC4 - 1))
                        nc.scalar.activation(qT[:, oc, s:s + 320], ps, AF.Identity,
                                             bias=Bp["bq"][:, oc:oc + 1])
            if "q" in g["dbg"]:
                for oc in range(NC4):
                    nc.sync.dma_start(g["dbg"]["q"][oc, :, :], qT[:, oc, :])

            # ---------------- flags ----------------
            mwt = sbB0.tile([L_T, WPC * WN], dt.float32)
            nc.sync.dma_start(mwt, g["mask_win"][:, :, :].rearrange("l w p -> l (w p)"))
            mx = sbB0.tile([L_T, WPC, 1], dt.float32)
            nc.vector.reduce_max(mx, mwt.rearrange("l (w p) -> l w p", w=WPC),
                                 axis=AX.X, opt_input=False, opt_output=False)
            with tc.tile_pool(name="flg_d", bufs=1, space="DRAM") as flgp:
                mx_d = flgp.tile([L_T, WPC], dt.float32)
                nc.sync.dma_start(mx_d, mx[:, :, 0])
                mrow = sbB0.tile([1, L_T * WPC], dt.float32)
                nc.sync.dma_start(mrow, mx_d[:, :].rearrange("l w -> (l w)")
                                  .unsqueeze(0))
            msum = sbB0.tile([1, WPC], dt.float32)
            nc.vector.tensor_add(msum, mrow[:, 0:WPC], mrow[:, WPC:2 * WPC])
            fl = sbB0.tile([1, WPC], dt.float32)
            nc.scalar.activation(fl, msum, AF.Sign)
            nc.sync.dma_start(g["flags_d"][:].unsqueeze(0), fl[0:1, :])
            if "flags" in g["dbg"]:
                nc.sync.dma_start(g["dbg"]["flags"][:].unsqueeze(0), fl[0:1, :])

            # ---------------- AllGather ----------------
            nc.gpsimd.collective_compute(
                "AllGather", mybir.AluOpType.bypass,
                ins=[g["contrib"][:, :]],
                outs=[g["agout"][:, :]],
                replica_groups=[list(range(N_CORES))],
            )

            # ================= stage B =================
            _stage_b(nc, tc, g, sbB0, ps_big, ps_mid, ps_y, W, Bp, Bf, qT, ident)


def _transpose_in(nc, ps_pool, sb_pool, src_ap, dst, npix, ident):
    """DMA pixel-major [npix, 512] DRAM -> transposed SBUF [128, 4, npix]."""
    for t in range((npix + 127) // 128):
        n = min(128, npix - t * 128)
        tmp = sb_pool.tile([128, DIM], dt.float32, tag="tr_in")
        nc.sync.dma_start(tmp[0:n, :], src_ap[t * 128:t * 128 + n, :])
        ps = ps_pool.tile([128, DIM], dt.float32, tag="big")
        for ic in range(NC4):
            nc.tensor.transpose(ps[:, ic * 128:ic * 128 + n],
                                tmp[0:n, ic * 128:(ic + 1) * 128], ident[0:n, 0:n])
        for ic in range(NC4):
            nc.vector.tensor_copy(dst[:, ic, t * 128:t * 128 + n],
                                  ps[:, ic * 128:ic * 128 + n])


def _stage_a(nc, tc, g, sb, ps_big, ps_mid, W, Bp, Bf, MW, pool_ind_t, hs, ident):
    HP = STRIP_H + 2  # 10 rows incl halo
    PIX01 = 2 * HP * WI  # 960 f01 pixels (with halo rows)
    PIX23 = 2 * STRIP_H * WI  # 768

    xs = g["x_strip"]
    maskb = sb.tile([128, PIX01], dt.float32)
    nc.sync.dma_start(maskb, g["mask_strip"][:, :, :].rearrange("l y x -> (l y x)")
                      .unsqueeze(0).to_broadcast([128, PIX01]))

    kT23 = sb.tile([128, NC4, PIX23], dt.float32)
    kmod_in = sb.tile([128, NC4, PIX01], dt.float32)  # k01+mask (mod input)
    vmod_in = sb.tile([128, NC4, PIX01], dt.float32)
    vc = g["contrib"][0, OFF_V:OFF_PK].rearrange("(t y x c) -> t y x c",
                                                 t=T, y=STRIP_H, x=64)

    def _v_out(vt, fr, t):
        f, y = divmod(t, 4)  # (frame offset, 2-row group)
        nc.sync.dma_start(vc[fr + f, 2 * y:2 * y + 2, 0:48, :], vt)
        nc.sync.dma_start(vc[fr + f, 2 * y, 48:64, :], vt[0:16, :])
        nc.sync.dma_start(vc[fr + f, 2 * y + 1, 48:64, :], vt[48:64, :])

    with tc.tile_pool(name="sbXT", bufs=1) as sbXT:
        # x^T over strip: f01 all 10 rows, f23 middle 8 rows
        xt01 = sbXT.tile([128, NC4, PIX01], dt.float32, tag="xt")
        _transpose_in(nc, ps_big, sbXT, xs[0:2].rearrange("t y x c -> (t y x) c"),
                      xt01, PIX01, ident)
        # k/v f01 projections straight into modulator-input tiles (+mask)
        _proj_T(nc, ps_big, W["wk"], Bp["bk"], xt01, kmod_in, PIX01, 320)
        _proj_T(nc, ps_big, W["wv"], Bp["bv"], xt01, vmod_in, PIX01, 320)
        for ic in range(NC4):
            nc.vector.tensor_add(kmod_in[:, ic, :], kmod_in[:, ic, :], maskb)
            nc.vector.tensor_add(vmod_in[:, ic, :], vmod_in[:, ic, :], maskb)

        xt23 = sbXT.tile([128, NC4, PIX01], dt.float32, tag="xt")
        _transpose_in(nc, ps_big, sbXT,
                      xs[2:4].rearrange("t y x c -> (t y x) c"),
                      xt23, PIX01, ident)
        for f in range(2):
            _proj_T(nc, ps_big, W["wk"], Bp["bk"],
                    xt23[:, :, f * 480 + 48:f * 480 + 432],
                    kT23[:, :, f * 384:(f + 1) * 384], 384, 384)
        for t in range(8):
            f, grp = t // 4, t % 4
            s = f * 480 + 48 + grp * 96
            ps = ps_big.tile([96, DIM], dt.float32, tag="big")
            for ic in range(NC4):
                nc.tensor.matmul(ps, xt23[:, ic, s:s + 96],
                                 W["wv"][:, ic, :], start=(ic == 0),
                                 stop=(ic == NC4 - 1))
            vst = sb.tile([96, DIM], dt.float32, tag="vst", name="vst")
            nc.vector.tensor_add(vst, ps, Bf["bv"][0:96, :])
            _v_out(vst, 2, t)

    # ---- modulators (replace f01) ----
    kT01m = _modulator(nc, tc, sb, ps_big, ps_mid, MW["k"], kmod_in, hs, "k")
    vT01m = _modulator(nc, tc, sb, ps_big, ps_mid, MW["v"], vmod_in, hs, "v")

    # ---- transpose v01m back to natural 96-pix tiles, stream out ----
    vT01f = vT01m.rearrange("p a t y x -> p a (t y x)")
    for t in range(8):  # (f, 2-row group)
        s = t * 96
        ps = ps_big.tile([96, DIM], dt.float32, tag="big")
        for ic in range(NC4):
            nc.tensor.transpose(ps[:, ic * 128:(ic + 1) * 128],
                                vT01f[:, ic, s:s + 96], ident)
        vst = sb.tile([96, DIM], dt.float32, tag="vst", name="vst")
        nc.vector.tensor_copy(vst, ps)
        _v_out(vst, 0, t)

    # ---- AG contributions ----
    kc = g["contrib"][0, OFF_K:OFF_V].rearrange("(a p t y x) -> p a t y x", a=NC4, p=128, t=T, y=STRIP_H)
    k01v = kT01m  # [128, 4, 2, 8, 48]
    k23v = kT23.rearrange("p a (t y x) -> p a t y x", t=2, y=STRIP_H)
    for ic in range(NC4):
        nc.sync.dma_start(kc[:, ic, 0:2, :, 0:48], k01v[:, ic])
        nc.sync.dma_start(kc[:, ic, 2:4, :, 0:48], k23v[:, ic])
        nc.sync.dma_start(kc[:, ic, 0:2, :, 48:64], k01v[:, ic, :, :, 0:16])
        nc.sync.dma_start(kc[:, ic, 2:4, :, 48:64], k23v[:, ic, :, :, 0:16])
    if "k_contrib" in g["dbg"]:
        nc.sync.dma_start(
            g["dbg"]["k_contrib"][:, :, :, :].rearrange("d t y x -> (d t y x)"),
            g["contrib"][0, OFF_K:OFF_V])
        nc.sync.dma_start(
            g["dbg"]["v_contrib"][:, :, :, :].rearrange("t y x d -> (t y x d)"),
            g["contrib"][0, OFF_V:OFF_PK])

    # ---- pooled tokens (72 rows on every core) ----
    px = sb.tile([PGRID, 6, DIM], dt.float32)  # [cell 12, cell-row 6, ch]
    for cr in range(6):
        ps = ps_big.tile([PGRID, DIM], dt.float32, tag="big")
        for h in range(2):  # two 2-row groups of the 4-row cell
            xrows = sb.tile([96, DIM], dt.float32, tag="xpoolrows")
            r0 = cr * 4 + h * 2
            nc.sync.dma_start(xrows, g["x_pool"][r0:r0 + 2].rearrange("y x c -> (y x) c"))
            nc.tensor.matmul(ps, pool_ind_t, xrows, start=(h == 0), stop=(h == 1))
        nc.vector.tensor_add(px[:, cr, :], ps, Bf["pool_b"][0:PGRID, :])
    # px^T [128,4,72]  (pooled-row order = (cell-row, cell), cr-major)
    pxT = sb.tile([128, NC4, POOL_PC], dt.float32)
    for ic in range(NC4):
        ps = ps_mid.tile([128, POOL_PC], dt.float32, tag="mid")
        for cr in range(6):
            nc.tensor.transpose(ps[:, cr * PGRID:(cr + 1) * PGRID],
                                px[:, cr, ic * 128:(ic + 1) * 128],
                                ident[0:PGRID, 0:PGRID])
        nc.vector.tensor_copy(pxT[:, ic, :], ps)
    # pk^T
    pkT = sb.tile([128, NC4, POOL_PC], dt.float32)
    for oc in range(NC4):
        ps2 = ps_mid.tile([128, POOL_PC], dt.float32, tag="mid")
        for ic in range(NC4):
            nc.tensor.matmul(ps2, W["wk"][:, ic, oc * 128:(oc + 1) * 128],
                             pxT[:, ic, :], start=(ic == 0), stop=(ic == NC4 - 1))
        nc.scalar.activation(pkT[:, oc, :], ps2, AF.Identity, bias=Bp["bk"][:, oc:oc + 1])
    nc.sync.dma_start(g["contrib"][0, OFF_PK:OFF_PV].rearrange("(a p n) -> p a n", a=NC4, p=128), pkT)
    # pv natural
    pv = sb.tile([POOL_PC, DIM], dt.float32)
    ps3 = ps_big.tile([POOL_PC, DIM], dt.float32, tag="big")
    for ic in range(NC4):
        nc.tensor.matmul(ps3, pxT[:, ic, 0:POOL_PC], W["wv"][:, ic, :],
                         start=(ic == 0), stop=(ic == NC4 - 1))
    nc.vector.tensor_add(pv, ps3, Bf["bv"][0:POOL_PC, :])
    nc.sync.dma_start(g["contrib"][0, OFF_PV:AG_TOT].rearrange("(n c) -> n c", n=POOL_PC), pv)
    if "pk" in g["dbg"]:
        nc.sync.dma_start(g["dbg"]["pk"][:, :].rearrange("d n -> (d n)"), g["contrib"][0, OFF_PK:OFF_PV])
        nc.sync.dma_start(g["dbg"]["pv"][:, :].rearrange("n d -> (n d)"), g["contrib"][0, OFF_PV:AG_TOT])


def _proj_T(nc, ps_pool, w, bias_p, xt, dst, npix, piece):
    """dst[:, oc, pix] = (W.T @ x^T)[oc chunk] + bias (transposed proj)."""
    n_p = (npix + piece - 1) // piece
    for oc in range(NC4):
        for p in range(n_p):
            s = p * piece
            e = min(npix, s + piece)
            ps = ps_pool.tile([128, piece], dt.float32, tag="big")
            for ic in range(NC4):
                nc.tensor.matmul(ps[:, 0:e - s], w[:, ic, oc * 128:(oc + 1) * 128],
                                 xt[:, ic, s:e], start=(ic == 0), stop=(ic == NC4 - 1))
            nc.scalar.activation(dst[:, oc, s:e], ps[:, 0:e - s], AF.Identity,
                                 bias=bias_p[:, oc:oc + 1])


def _modulator(nc, tc, sb, ps_big, ps_mid, mw, mod_in, hs, tag):
    """Modulator on transposed f01 (k|v)+mask data [128, 4, 960] (10 rows
    incl halo). Returns modulated transposed [128, 4, 960] (rows 1..9 valid)."""
    HP = STRIP_H + 2
    PIX01 = 2 * HP * WI
    outT = sb.tile([128, NC4, 2, STRIP_H, WI], dt.float32, tag=f"modo{tag}")
    with tc.tile_pool(name=f"sbM{tag}", bufs=1) as sbm:
        # conv1 1x1 512->128 (+bias) + leaky relu -> padded rows [128,2,10,50]
        lx1 = sbm.tile([128, 2, HP, 50], dt.float32, tag="lx1")
        nc.vector.memset(lx1, 0.0)
        for f in range(2):
            s = f * (HP * WI)
            ps = ps_mid.tile([128, HP * WI], dt.float32, tag="mid")
            for ic in range(NC4):
                nc.tensor.matmul(ps, mw["sq"][:, ic, :],
                                 mod_in[:, ic, s:s + HP * WI],
                                 start=(ic == 0), stop=(ic == NC4 - 1))
            nc.scalar.activation(lx1[:, f, :, 1:49],
                                 ps.rearrange("p (y x) -> p y x", y=HP),
                                 AF.Identity, bias=mw["sqb"][:, 0:1])
            lint = lx1[:, f, :, 1:49]
            ltmp = sbm.tile([128, HP, 48], dt.float32, tag="ltmp", name="ltmp")
            nc.vector.tensor_scalar_mul(ltmp, lint, 0.2)
            nc.vector.tensor_max(lint, lint, ltmp)
        # zero the halo rows at image edges (conv zero-padding semantics)
        for f in range(2):
            nc.vector.tensor_scalar_mul(lx1[:, f, 0, :], lx1[:, f, 0, :],
                                        hs[:, 0:1])
            nc.vector.tensor_scalar_mul(lx1[:, f, HP - 1, :],
                                        lx1[:, f, HP - 1, :], hs[:, 1:2])
        # conv2 3x3 128->128 (+bias) + SiLU -> lx2 [128, 2, 384]
        lx2 = sbm.tile([128, 2, STRIP_H * WI], dt.float32, tag="lx2")
        for f in range(2):
            ps = ps_mid.tile([128, STRIP_H * WI], dt.float32, tag="mid")
            ti = 0
            for dy in (-1, 0, 1):
                for dx in (-1, 0, 1):
                    rhs = lx1[:, f, 1 + dy:1 + dy + STRIP_H, 1 + dx:1 + dx + WI]
                    nc.tensor.matmul(ps, mw["f"][:, ti, :], rhs,
                                     start=(ti == 0), stop=(ti == 8))
                    ti += 1
            sg = sbm.tile([128, STRIP_H * WI], dt.float32, tag="modsg")
            nc.scalar.activation(sg, ps, AF.Sigmoid, bias=mw["fb"][:, 0:1])
            tmp = sbm.tile([128, STRIP_H * WI], dt.float32, tag="modt")
            nc.scalar.activation(tmp, ps, AF.Identity, bias=mw["fb"][:, 0:1])
            nc.vector.tensor_mul(lx2[:, f, :], tmp, sg)
        # conv3 1x1 128->512 + bias -> transposed tile (strip rows 1..9)
        for f in range(2):
            for oc in range(NC4):
                ps = ps_mid.tile([128, STRIP_H * WI], dt.float32, tag="mid")
                nc.tensor.matmul(ps, mw["un"][:, oc * 128:(oc + 1) * 128],
                                 lx2[:, f, :], start=True, stop=True)
                nc.scalar.activation(outT[:, oc, f],
                                     ps.rearrange("p (y x) -> p y x", y=8),
                                     AF.Identity, bias=mw["unb"][:, oc:oc + 1])
    return outT


def _stage_b(nc, tc, g, sb0, ps_big, ps_mid, ps_y, W, Bp, Bf, qT, ident):
    ag = g["agout"]
    kgv = ag[:, OFF_K:OFF_V].rearrange("s (a p t y x) -> p s a t y x",
                                       a=NC4, p=128, t=T, y=STRIP_H)
    vgv = ag[:, OFF_V:OFF_PK].rearrange("s (t y x c) -> s t y x c",
                                        t=T, y=STRIP_H, x=64)
    pkgv = ag[:, OFF_PK:OFF_PV].rearrange("s (a p n) -> p s a n", a=NC4, p=128)
    pvgv = ag[:, OFF_PV:AG_TOT].rearrange("s (n c) -> s n c", n=POOL_PC)

    with (
        tc.tile_pool(name="sbB", bufs=2) as sb,
        tc.tile_pool(name="sbBig", bufs=1) as sbig,
        tc.tile_pool(name="sbP", bufs=1) as sbP,
        tc.tile_pool(name="dramp", bufs=4, space="DRAM") as dramp,
    ):
        # pooled keys resident: pk_sb [128, 4, 8, 72], pv_sb [72, 8, 512]
        pk_sb = sbP.tile([128, NC4, N_CORES, POOL_PC], dt.float32)
        for oc in range(NC4):
            nc.sync.dma_start(pk_sb[:, oc, :, :], pkgv[:, :, oc, :])
        pv_sb = sbP.tile([POOL_PC, N_CORES, DIM], dt.float32)
        for s in range(N_CORES):
            nc.sync.dma_start(pv_sb[:, s, :], pvgv[s])

        # dynamic-offset registers (per window: prev,cur,next,x0,x_own)
        meta_t = sbP.tile([1, META_LEN], dt.int32)
        nc.sync.dma_start(meta_t, g["meta"][:].unsqueeze(0))
        regs = []
        for i in range(WPC * 5):
            r = nc.alloc_register(mybir.EngineType.SP, f"mreg{i}")
            nc.sync.reg_load(r, meta_t[0:1, i:i + 1])
            regs.append(nc.sync.snap(r))

        for wi in range(WPC):
            r_prev, r_cur, r_next, r_x0, r_xo = regs[wi * 5:wi * 5 + 5]
            ds = bass.ds
            # ---- k patch [128, 4(oc), 4(piece), 4(f), 64(y4 x16)] ----
            kp = sbig.tile([128, NC4, 4, T, 64], dt.float32, tag="kp")
            pieces = [(r_prev, 4), (r_cur, 0), (r_cur, 4), (r_next, 0)]
            for oc in range(NC4):
                for pi, (slot, y0) in enumerate(pieces):
                    nc.sync.dma_start(
                        kp[:, oc, pi].rearrange("p t (y x) -> p t y x", y=4),
                        kgv[:, ds(slot, 1), oc, :, y0:y0 + 4, ds(r_x0, 16)]
                           .squeeze(1))
            # ---- v patch [128, 8, 512]: 16 (piece,f) blocks of 64 pix ----
            vp = sbig.tile([128, 8, DIM], dt.float32, tag="vp")
            for pi, (slot, y0) in enumerate(pieces):
                for f in range(T):
                    b = pi * 4 + f
                    nc.sync.dma_start(
                        vp[64 * (b % 2):64 * (b % 2) + 64, b // 2, :],
                        vgv[ds(slot, 1), f, y0:y0 + 4, ds(r_x0, 16), :]
                           .squeeze(0))
            # ---- v own-window [64, 4, 512] ----
            vown = sbig.tile([64, T, DIM], dt.float32, tag="vown")
            for f in range(T):
                nc.sync.dma_start(
                    vown[:, f, :],
                    vgv[ds(r_cur, 1), f, :, ds(r_xo, 8), :]
                       .squeeze(0))
            # ---- k^T own-window [128, 4(oc), 4(f), 64] ----
            kown = sbig.tile([128, NC4, T, WN], dt.float32, tag="kown")
            for oc in range(NC4):
                nc.sync.dma_start(
                    kown[:, oc].rearrange("p f (y x) -> p f y x", y=WH),
                    kgv[:, ds(r_cur, 1), oc, :, :, ds(r_xo, 8)].squeeze(1))
            # ---- flag bcast [128,1] ----
            flb = sb.tile([128, 1], dt.float32, tag="flb")
            nc.sync.dma_start(flb, g["flags_d"][wi:wi + 1].unsqueeze(0)
                              .to_broadcast([128, 1]))

            q_w = qT[:, :, wi * NQ:(wi + 1) * NQ]  # [128, 4, 256]
            kpf = kp.rearrange("p a x t n -> p a (x t n)")  # [128,4,1024]
            yfin = sbig.tile([128, N_HEAD, NQ], dt.float32, tag="yfin")

            for h in range(N_HEAD):
                # ================= global attention =================
                pT = sbig.tile([128, NCHUNK, NQ], dt.float32, tag="pT")
                for j in range(NCHUNK):
                    n = KCH[j]
                    ps = ps_mid.tile([128, NQ], dt.float32, tag="mid")
                    if j < 8:
                        lhs = kpf[:, h, j * 128:(j + 1) * 128]
                    else:
                        lhs = pk_sb[:, h, j - 8, :]
                    nc.tensor.matmul(ps[0:n, :], lhs, q_w[:, h, :],
                                     start=True, stop=True)
                    nc.scalar.activation(pT[0:n, j, :], ps[0:n, :], AF.Exp,
                                         scale=SCALE)
                # denominators: p_acc -> transpose -> row sums -> recip row
                p_acc = sb.tile([128, NQ], dt.float32, tag="pacc")
                nc.vector.tensor_add(p_acc, pT[:, 0, :], pT[:, 1, :])
                for j in range(2, 8):
                    nc.vector.tensor_add(p_acc, p_acc, pT[:, j, :])
                for j in range(8, NCHUNK):
                    nc.vector.tensor_add(p_acc[0:POOL_PC, :], p_acc[0:POOL_PC, :],
                                         pT[0:POOL_PC, j, :])
                rrow_g = _recip_row(nc, sb, ps_mid, p_acc, ident, "g", dram=dramp)
                # AV accumulate
                psy = ps_y.tile([128, NQ], dt.float32, tag="y")
                for j in range(NCHUNK):
                    n = KCH[j]
                    if j < 8:
                        lhs = vp[0:n, j, h * 128:(h + 1) * 128]
                    else:
                        lhs = pv_sb[:, j - 8, h * 128:(h + 1) * 128]
                    nc.tensor.matmul(psy, lhs, pT[0:n, j, :],
                                     start=(j == 0), stop=(j == NCHUNK - 1))
                rgB = sb.tile([128, NQ], dt.float32, tag="rgB", name="rgB")
                nc.gpsimd.dma_start(rgB, rrow_g[:, :].to_broadcast([128, NQ]))
                y_g = sb.tile([128, NQ], dt.float32, tag="yg_sb")
                nc.vector.tensor_mul(y_g, psy, rgB)

                # ================= local attention =================
                psl = ps_mid.tile([64, NQ], dt.float32, tag="mid")
                for f in range(T):
                    nc.tensor.matmul(psl[:, f * WN:(f + 1) * WN],
                                     kown[:, h, f, :],
                                     q_w[:, h, f * WN:(f + 1) * WN],
                                     start=True, stop=True)
                ploc = sb.tile([64, NQ], dt.float32, tag="ploc")
                nc.scalar.activation(ploc, psl, AF.Exp, scale=SCALE)
                rrow_l = _recip_row(nc, sb, ps_mid, ploc, ident, "l", dram=dramp, parts=64)
                psyl = ps_y.tile([128, NQ], dt.float32, tag="y")
                for f in range(T):
                    # own-window keys for frame f in ploc rows: order (piece,y,x8)
                    nc.tensor.matmul(psyl[:, f * WN:(f + 1) * WN],
                                     vown[:, f, h * 128:(h + 1) * 128],
                                     ploc[:, f * WN:(f + 1) * WN],
                                     start=True, stop=True)
                rlB = sb.tile([128, NQ], dt.float32, tag="rlB", name="rlB")
                nc.gpsimd.dma_start(rlB, rrow_l[:, :].to_broadcast([128, NQ]))
                y_l = sb.tile([128, NQ], dt.float32, tag="yl_sb")
                nc.vector.tensor_mul(y_l, psyl, rlB)

                # ---- blend: y = y_l + flag*(y_g - y_l) ----
                dlt = sb.tile([128, NQ], dt.float32, tag="dlt")
                nc.vector.tensor_sub(dlt, y_g, y_l)
                nc.vector.tensor_scalar_mul(dlt, dlt, flb[:, 0:1])
                nc.vector.tensor_add(yfin[:, h, :], y_l, dlt)

            # ================= output projection =================
            for fp in range(2):
                pso = ps_big.tile([128, DIM], dt.float32, tag="big")
                for h in range(N_HEAD):
                    nc.tensor.matmul(pso, yfin[:, h, fp * 128:(fp + 1) * 128],
                                     W["wp"][:, h, :],
                                     start=(h == 0), stop=(h == N_HEAD - 1))
                osb = sb.tile([128, DIM], dt.float32, tag="osb")
                nc.vector.tensor_add(osb, pso, Bf["bp"])
                nc.sync.dma_start(
                    g["out_win"][wi, 2 * fp:2 * fp + 2, :, :]
                        .rearrange("t p c -> (t p) c"), osb)


def _recip_row(nc, sb, ps_mid, p_acc, ident, tag, dram=None, parts=128):
    """sum over partitions of p_acc[parts, 256] -> reciprocal -> [1,256] row."""
    sums = sb.tile([128, 2], dt.float32, tag=f"sum{tag}")
    for half in range(2):
        ps = ps_mid.tile([128, 128], dt.float32, tag="mid")
        nc.tensor.transpose(ps[:, 0:parts],
                            p_acc[0:parts, half * 128:(half + 1) * 128],
                            ident[0:parts, 0:parts])
        nc.vector.reduce_sum(sums[:, half:half + 1], ps[:, 0:parts], axis=AX.X)
    rec = sb.tile([128, 2], dt.float32, tag=f"rec{tag}")
    nc.vector.reciprocal(rec, sums)
    rd = dram.tile([1, NQ], dt.float32, tag="rrow_d", name="rrow_d")
    nc.sync.dma_start(rd[0:1, 0:128], rec[:, 0:1])
    nc.sync.dma_start(rd[0:1, 128:256], rec[:, 1:2])
    return rd


# ==================== host side ====================

def _host_inputs(inputs, debug=False):
    x = np.asarray(inputs["x"], np.float32)[0]  # [4,48,48,512]
    mask = np.asarray(inputs["mask"], np.float32)[0, :, :, :, 0]  # [2,48,48]

    pool_ind = np.zeros((2 * WI, PGRID), np.float32)
    for y in range(2):
        for xx in range(WI):
            pool_ind[y * WI + xx, xx // PW] = 1.0 / (PH * PW)

    common = dict(
        wq=np.asarray(inputs["Wq"], np.float32), bq=np.asarray(inputs["bq"], np.float32),
        wk=np.asarray(inputs["Wk"], np.float32), bk=np.asarray(inputs["bk"], np.float32),
        wv=np.asarray(inputs["Wv"], np.float32), bv=np.asarray(inputs["bv"], np.float32),
        wp=np.asarray(inputs["Wp"], np.float32), bp=np.asarray(inputs["bp"], np.float32),
        pool_b=np.asarray(inputs["pool_b"], np.float32),
        pool_ind=pool_ind,
    )
    for tag, pre in (("k", "kmod"), ("v", "vmod")):
        common[f"{tag}sq"] = np.ascontiguousarray(
            np.asarray(inputs[f"{pre}_sq_w"], np.float32)[:, :, 0, 0].T)
        common[f"{tag}sqb"] = np.asarray(inputs[f"{pre}_sq_b"], np.float32)
        fw = np.asarray(inputs[f"{pre}_f_w"], np.float32)
        common[f"{tag}f"] = np.ascontiguousarray(
            np.stack([fw[:, :, dy, dx].T for dy in range(3) for dx in range(3)]))
        common[f"{tag}fb"] = np.asarray(inputs[f"{pre}_f_b"], np.float32)
        common[f"{tag}un"] = np.ascontiguousarray(
            np.asarray(inputs[f"{pre}_un_w"], np.float32)[:, :, 0, 0].T)
        common[f"{tag}unb"] = np.asarray(inputs[f"{pre}_un_b"], np.float32)

    in_maps = []
    for c in range(N_CORES):
        m = dict(common)
        # strip rows with halo
        if c < N_STRIP:
            r0 = c * STRIP_H
            xs = np.zeros((T, STRIP_H + 2, WI, DIM), np.float32)
            ms = np.zeros((L_T, STRIP_H + 2, WI), np.float32)
            lo, hi = max(0, r0 - 1), min(HI, r0 + STRIP_H + 1)
            xs[:, lo - (r0 - 1):lo - (r0 - 1) + hi - lo] = x[:, lo:hi]
            ms[:, lo - (r0 - 1):lo - (r0 - 1) + hi - lo] = mask[:, lo:hi]
            m["x_strip"] = xs
            m["mask_strip"] = ms
            m["halo_scale"] = np.array(
                [0.0 if r0 == 0 else 1.0,
                 0.0 if r0 + STRIP_H == HI else 1.0], np.float32)
        else:
            m["x_strip"] = np.zeros((T, STRIP_H + 2, WI, DIM), np.float32)
            m["mask_strip"] = np.zeros((L_T, STRIP_H + 2, WI), np.float32)
            m["halo_scale"] = np.ones(2, np.float32)
        # window inputs
        xw = np.zeros((WPC, T, WN, DIM), np.float32)
        mw = np.zeros((L_T, WPC, WN), np.float32)
        for k, w in enumerate(WIN_ASSIGN[c]):
            i, j = w // N_WW, w % N_WW
            blk = x[:, 8 * i:8 * i + 8, 8 * j:8 * j + 8, :]
            xw[k] = blk.reshape(T, WN, DIM)
            mw[:, k] = mask[:, 8 * i:8 * i + 8, 8 * j:8 * j + 8].reshape(L_T, WN)
        m["x_win"] = xw
        m["mask_win"] = mw
        # pool rows: 72 pooled cells = frame c//2, cell-rows 6*(c%2)..+6
        f, pr0 = c // 2, 6 * (c % 2)
        m["x_pool"] = np.ascontiguousarray(x[f, pr0 * 4:pr0 * 4 + 24])
        m["meta"] = _meta_for_core(c)
        in_maps.append(m)
    return in_maps


def _get_nc(debug=False):
    key = bool(debug)
    if key not in _NC_CACHE:
        _NC_CACHE[key] = build_nc(debug=debug)
    return _NC_CACHE[key]


def run_spmd(inputs, debug=False):
    nc = _get_nc(debug=debug)
    in_maps = _host_inputs(inputs, debug=debug)
    res = run_bass_kernel_spmd(nc, in_maps, list(range(N_CORES)))
    return res


def assemble(results):
    out = np.zeros((T, HI, WI, DIM), np.float32)
    done = set()
    for c in range(N_CORES):
        ow = results[c]["out_win"]  # [5,4,64,512]
        for k, w in enumerate(WIN_ASSIGN[c]):
            if w in done:
                continue
            done.add(w)
            i, j = w // N_WW, w % N_WW
            out[:, 8 * i:8 * i + 8, 8 * j:8 * j + 8, :] = \
                ow[k].reshape(T, WH, WW, DIM)
    return out[None]


def kernel(**inputs):
    res = run_spmd(inputs)
    return assemble(res.results)


_CALLABLE_CACHE = {}


def _get_callable(debug=False):
    """Build the sharded jitted callable once (mirrors run_bass_via_pjrt)."""
    key = bool(debug)
    if key in _CALLABLE_CACHE:
        return _CALLABLE_CACHE[key]
    import jax
    from jax.sharding import Mesh, PartitionSpec
    from jax.experimental.shard_map import shard_map
    from concourse import bass2jax, mybir as _mb

    nc = _get_nc(debug=debug)
    bass2jax.install_neuronx_cc_hook()
    in_names, out_names, out_avals, zero_outs = [], [], [], []
    pname = nc.partition_id_tensor.name if nc.partition_id_tensor else None
    for alloc in nc.m.functions[0].allocations:
        if not isinstance(alloc, _mb.MemoryLocationSet):
            continue
        name = alloc.memorylocations[0].name
        if alloc.kind == "ExternalInput":
            if name != pname:
                in_names.append(name)
        elif alloc.kind == "ExternalOutput":
            out_names.append(name)
            shape = tuple(alloc.tensor_shape)
            dtp = _mb.dt.np(alloc.dtype)
            out_avals.append(jax.core.ShapedArray(shape, dtp))
            zero_outs.append(np.zeros(shape, dtp))
    n_params = len(in_names)
    all_in = list(in_names) + list(out_names)
    if pname is not None:
        all_in.append(pname)

    def _body(*args):
        ops = list(args)
        if pname is not None:
            ops.append(bass2jax.partition_id_tensor())
        return tuple(bass2jax._bass_exec_p.bind(
            *ops, out_avals=tuple(out_avals), in_names=tuple(all_in),
            out_names=tuple(out_names), lowering_input_output_aliases=(),
            sim_require_finite=True, sim_require_nnan=True, nc=nc))

    devices = jax.devices()[:N_CORES]
    mesh = Mesh(np.asarray(devices), ("core",))
    n_outs = len(out_names)
    sharded = jax.jit(
        shard_map(_body, mesh=mesh,
                  in_specs=(PartitionSpec("core"),) * (n_params + n_outs),
                  out_specs=(PartitionSpec("core"),) * n_outs,
                  check_rep=False),
        donate_argnums=tuple(range(n_params, n_params + n_outs)),
        keep_unused=True)
    info = (sharded, in_names, out_names, out_avals, zero_outs)
    _CALLABLE_CACHE[key] = info
    return info


def timed_run(inputs, iters=4, debug=False):
    """Run via a cached jitted callable; returns (results, best_wall_s)."""
    import time as _time
    import jax
    sharded, in_names, out_names, out_avals, zero_outs = _get_callable(debug)
    in_maps = _host_inputs(inputs, debug=debug)
    dbgz = np.zeros((1, 2), np.uint32)  # dbg_addr placeholder (debug builds)
    concat_in = [np.concatenate(
        [np.asarray(in_maps[c].get(n, dbgz)) for c in range(N_CORES)], 0)
        for n in in_names]
    concat_in = [jax.device_put(a) for a in concat_in]
    best = None
    out_arrs = None
    for _ in range(iters):
        zeros = [np.zeros((N_CORES * z.shape[0],) + z.shape[1:], z.dtype)
                 for z in zero_outs]
        t0 = _time.perf_counter()
        out_arrs = sharded(*concat_in, *zeros)
        jax.block_until_ready(out_arrs)
        dt_ = _time.perf_counter() - t0
        best = dt_ if best is None else min(best, dt_)
    results = [
        {n: np.asarray(out_arrs[i]).reshape(N_CORES, *out_avals[i].shape)[c]
         for i, n in enumerate(out_names)}
        for c in range(N_CORES)
    ]
    return results, best



# revision 20
# speedup vs baseline: 2.8625x; 2.8625x over previous
"""Trainium2 Bass kernel for nn_ErrorAwareSelfAttention (8 NeuronCores).

v2 design (bf16 matmul pipeline):
- All heavy matmuls run in bf16 (1 cyc/row vs fp32's 4) with fp32 PSUM
  accumulation; host pre-converts x / weights to bf16.
- Stage A (strips of 8 image rows on cores 0-5, SPMD on all):
  x^T via XBAR dma-transpose straight from DRAM; k/v projections;
  Modulator convs on frames 0,1 (Lrelu/Silu on ACT); k and v both
  written to DRAM contribs in NATURAL [t, y, x64, ch] layout (ch
  contiguous -> 1KB DMA runs); pooled tokens (pk^T, pv) sharded over
  all 8 cores; one bf16 AllGather publishes everything.
- Stage B (5 windows per core): the 16x16 halo patch of k and v is
  gathered in natural layout (5 DMAs each, own-pixels-first ordering),
  k patch transposed on-chip by one XBAR DMA; scores per head in 13
  key chunks (8 patch + 5 pooled); exp on ACT in groups of 4 chunks
  (PSUM-bank sized); transposed AV produces y^T [q, ch] so softmax
  denominators are per-PARTITION scalars: denoms via tiny ap=1
  ones-matmuls on PE, normalization via DVE tensor_scalar; the local
  (unmasked) path reuses the global exp values (own-window keys are
  partitions 0:64 of even chunks) - no second exp; blend by per-window
  mask flag; y^T transposed back by XBAR for the output projection.
"""

import math
import sys

sys.path.insert(0, "/opt/trn_rl_repo")

import numpy as np
import ml_dtypes

import concourse.bass as bass
import concourse.mybir as mybir
import concourse.tile as tile
from concourse import bacc
from concourse.bass_utils import run_bass_kernel_spmd

dt = mybir.dt
AF = mybir.ActivationFunctionType
AX = mybir.AxisListType
ALU = mybir.AluOpType
BF = ml_dtypes.bfloat16

# ---------------- problem constants (hardcoded) ----------------
DIM = 512
N_HEAD = 4
CH = 128
WH = WW = 8
EH = EW = 4
PH = PW = 4
B, T, HI, WI = 1, 4, 48, 48
L_T = 2
N_WH = N_WW = 6
NW = 36
WN = 64
SCALE = 1.0 / math.sqrt(CH)
N_CORES = 8
NC4 = 4  # 512 / 128 channel chunks

STRIP_H = 8
N_STRIP = 6
HP = STRIP_H + 2          # strip rows incl 1-row halo
PIX01 = 2 * HP * WI       # 960
WPC = 5
_bounds = [int(NW * c / N_CORES) for c in range(N_CORES + 1)]
WIN_ASSIGN = []
for c in range(N_CORES):
    ws = list(range(_bounds[c], _bounds[c + 1]))
    while len(ws) < WPC:
        ws.append(ws[-1])
    WIN_ASSIGN.append(ws)

PGRID = HI // PH          # 12
NPOOL = T * PGRID * PGRID  # 576
POOL_PC = NPOOL // N_CORES  # 72
NPCH = 5                  # pool key chunks: 4x128 + 64
NQ = T * WN               # 256 queries / window
NCHUNK = 13               # 8 patch chunks (128 keys) + 5 pool chunks
EXP_GROUPS = [(0, 4), (4, 8), (8, 12), (12, 13)]

# AllGather packing (bf16 elements)
OFF_K = 0
OFF_V = OFF_K + T * STRIP_H * 64 * DIM
OFF_PK = OFF_V + T * STRIP_H * 64 * DIM
OFF_PV = OFF_PK + 128 * NC4 * POOL_PC
AG_TOT = OFF_PV + POOL_PC * DIM
AG_OUT_BYTES = N_CORES * AG_TOT * 2

REGS_PER_WIN = 6  # prev, cur, next, x0, xo, x12

_NC_CACHE = {}


def _meta_for_core(c):
    vals = []
    for w in WIN_ASSIGN[c]:
        i, j = w // N_WW, w % N_WW
        x0 = (8 * j - 4) % 48
        vals += [(i - 1) % N_STRIP, i, (i + 1) % N_STRIP, x0, 8 * j, x0 + 12]
    pad = (-len(vals)) % 32
    vals += [0] * pad
    return np.asarray(vals, np.int32)


META_LEN = len(_meta_for_core(0))


def build_nc(debug=False):
    nc = bacc.Bacc("TRN2", target_bir_lowering=False, debug=True)

    # ---------------- I/O ----------------
    x_strip = nc.dram_tensor("x_strip", [T, HP, WI, DIM], dt.bfloat16,
                             kind="ExternalInput")
    x_win = nc.dram_tensor("x_win", [WPC, T, WN, DIM], dt.bfloat16,
                           kind="ExternalInput")
    x_pool = nc.dram_tensor("x_pool", [24, WI, DIM], dt.bfloat16,
                            kind="ExternalInput")
    mask_strip = nc.dram_tensor("mask_strip", [L_T, HP, WI], dt.bfloat16,
                                kind="ExternalInput")
    mask_win = nc.dram_tensor("mask_win", [L_T, WPC, WN], dt.float32,
                              kind="ExternalInput")
    halo_scale = nc.dram_tensor("halo_scale", [2], dt.float32,
                                kind="ExternalInput")
    meta = nc.dram_tensor("meta", [META_LEN], dt.int32, kind="ExternalInput")
    pool_ind = nc.dram_tensor("pool_ind", [128, 9, POOL_PC], dt.bfloat16,
                              kind="ExternalInput")

    wtens = {}
    for nm in ("wq", "wk", "wv", "wp"):
        wtens[nm] = nc.dram_tensor(nm, [DIM, DIM], dt.bfloat16,
                                   kind="ExternalInput")
    for nm in ("bq", "bk", "bv", "bp", "pool_b"):
        wtens[nm] = nc.dram_tensor(nm, [DIM], dt.float32,
                                   kind="ExternalInput")
    mods = {}
    for tag in ("k", "v"):
        mods[tag] = dict(
            sq=nc.dram_tensor(f"{tag}sq", [DIM, 128], dt.bfloat16, kind="ExternalInput"),
            sqb=nc.dram_tensor(f"{tag}sqb", [128], dt.float32, kind="ExternalInput"),
            f=nc.dram_tensor(f"{tag}f", [9, 128, 128], dt.bfloat16, kind="ExternalInput"),
            fb=nc.dram_tensor(f"{tag}fb", [128], dt.float32, kind="ExternalInput"),
            un=nc.dram_tensor(f"{tag}un", [128, DIM], dt.bfloat16, kind="ExternalInput"),
            unb=nc.dram_tensor(f"{tag}unb", [DIM], dt.float32, kind="ExternalInput"),
        )

    out_win = nc.dram_tensor("out_win", [WPC, T, WN, DIM], dt.float32,
                             kind="ExternalOutput")
    dbg = {}
    if debug:
        dbg["k"] = nc.dram_tensor("dbg_k", [STRIP_H, 64, T, DIM], dt.bfloat16,
                                  kind="ExternalOutput")
        dbg["v"] = nc.dram_tensor("dbg_v", [STRIP_H, 64, T, DIM], dt.bfloat16,
                                  kind="ExternalOutput")
        dbg["pk"] = nc.dram_tensor("dbg_pk", [128, NC4, POOL_PC], dt.bfloat16,
                                   kind="ExternalOutput")
        dbg["pv"] = nc.dram_tensor("dbg_pv", [POOL_PC, DIM], dt.bfloat16,
                                   kind="ExternalOutput")
        dbg["q"] = nc.dram_tensor("dbg_q", [128, NC4, WPC * NQ], dt.bfloat16,
                                  kind="ExternalOutput")
        dbg["flags"] = nc.dram_tensor("dbg_flags", [128, WPC], dt.float32,
                                      kind="ExternalOutput")
        dbg["kp"] = nc.dram_tensor("dbg_kp", [128, 8, DIM], dt.bfloat16,
                                   kind="ExternalOutput")
        dbg["vp"] = nc.dram_tensor("dbg_vp", [128, 8, DIM], dt.bfloat16,
                                   kind="ExternalOutput")
        dbg["kpT"] = nc.dram_tensor("dbg_kpT", [128, 32, 128], dt.bfloat16,
                                    kind="ExternalOutput")
        dbg["pT"] = nc.dram_tensor("dbg_pT", [128, NCHUNK, NQ], dt.bfloat16,
                                   kind="ExternalOutput")

    contrib = nc.dram_tensor("contrib", [1, AG_TOT], dt.bfloat16)
    agout = nc.dram_tensor("agout", [N_CORES, AG_TOT], dt.bfloat16,
                           addr_space="Shared")

    g = dict(locals())
    g.update(wtens)
    with tile.TileContext(nc, num_cores=N_CORES) as tc:
        _program(nc, tc, g)
    nc.compile()
    return nc


def _program(nc, tc, g):
    dbg = g["dbg"]
    with (
        tc.tile_pool(name="wpool", bufs=1) as wpool,
        tc.tile_pool(name="persist", bufs=1) as sbP,
    ):
        # ---- weights / consts ----
        W = {}
        for nm in ("wq", "wk", "wv", "wp"):
            t = wpool.tile([128, NC4, DIM], dt.bfloat16, tag=nm, name=nm)
            nc.sync.dma_start(t, g[nm][:, :].rearrange("(a p) o -> p a o", p=128))
            W[nm] = t
        Bp = {}
        for nm in ("bq", "bk", "bv", "pool_b"):
            t = wpool.tile([128, NC4], dt.float32, tag=nm + "p", name=nm + "p")
            nc.sync.dma_start(t, g[nm][:].rearrange("(a p) -> p a", p=128))
            Bp[nm] = t
        Bf = {}
        for nm in ("bk", "bv", "bp", "pool_b"):
            t = wpool.tile([128, DIM], dt.float32, tag=nm + "f", name=nm + "f")
            nc.sync.dma_start(t, g[nm][:].unsqueeze(0).to_broadcast([128, DIM]))
            Bf[nm] = t
        MW = {}
        for tag in ("k", "v"):
            m = g["mods"][tag]
            MW[tag] = dict(
                sq=wpool.tile([128, NC4, 128], dt.bfloat16, tag=f"{tag}sqw", name=f"{tag}sqw"),
                sqb=wpool.tile([128, 1], dt.float32, tag=f"{tag}sqbw", name=f"{tag}sqbw"),
                f=wpool.tile([128, 9, 128], dt.bfloat16, tag=f"{tag}fw", name=f"{tag}fw"),
                fb=wpool.tile([128, 1], dt.float32, tag=f"{tag}fbw", name=f"{tag}fbw"),
                un=wpool.tile([128, DIM], dt.bfloat16, tag=f"{tag}unw", name=f"{tag}unw"),
                unb=wpool.tile([128, NC4], dt.float32, tag=f"{tag}unbw", name=f"{tag}unbw"),
            )
            nc.sync.dma_start(MW[tag]["sq"], m["sq"][:, :].rearrange("(a p) o -> p a o", p=128))
            nc.sync.dma_start(MW[tag]["sqb"], m["sqb"][:].unsqueeze(1))
            nc.sync.dma_start(MW[tag]["f"], m["f"][:, :, :].rearrange("n p o -> p n o"))
            nc.sync.dma_start(MW[tag]["fb"], m["fb"][:].unsqueeze(1))
            nc.sync.dma_start(MW[tag]["un"], m["un"][:, :])
            nc.sync.dma_start(MW[tag]["unb"], m["unb"][:].rearrange("(a p) -> p a", p=128))
        pool_ind_t = wpool.tile([128, 9, POOL_PC], dt.bfloat16)
        nc.sync.dma_start(pool_ind_t, g["pool_ind"][:, :, :])
        hs = wpool.tile([128, 2], dt.float32)
        nc.sync.dma_start(hs, g["halo_scale"][:].unsqueeze(0).to_broadcast([128, 2]))
        ones_b = wpool.tile([128, 1], dt.bfloat16)
        nc.vector.memset(ones_b, 1.0)
        ones2 = wpool.tile([2, 1], dt.float32)
        nc.vector.memset(ones2, 1.0)
        ones1 = wpool.tile([1, 128], dt.float32)
        nc.vector.memset(ones1, 1.0)

        # persistent across stages
        qT = sbP.tile([128, NC4, WPC * NQ], dt.bfloat16)
        flb = sbP.tile([128, WPC], dt.float32)

        # ================= stage A =================
        with (
            tc.tile_pool(name="psA", bufs=4, space="PSUM") as psA,
            tc.tile_pool(name="psA2", bufs=2, space="PSUM") as psA2,
            tc.tile_pool(name="sbA", bufs=1) as sbA,
        ):
            _stage_a(nc, tc, g, sbA, psA, psA2, W, Bp, Bf, MW, pool_ind_t,
                     hs, ones2, ones1, qT, flb, dbg)

        # ---------------- AllGather ----------------
        nc.gpsimd.collective_compute(
            "AllGather", mybir.AluOpType.bypass,
            ins=[g["contrib"][:, :]],
            outs=[g["agout"][:, :]],
            replica_groups=[list(range(N_CORES))],
        )

        # ================= stage B =================
        with (
            tc.tile_pool(name="ps_s", bufs=2, space="PSUM") as ps_s,
            tc.tile_pool(name="ps_y", bufs=2, space="PSUM") as ps_y,
            tc.tile_pool(name="ps_d", bufs=1, space="PSUM") as ps_d,
            tc.tile_pool(name="ps_p", bufs=1, space="PSUM") as ps_p,
            tc.tile_pool(name="sbB", bufs=2) as sbB,
            tc.tile_pool(name="sbB1", bufs=1) as sbB1,
        ):
            _stage_b(nc, tc, g, sbB, sbB1, ps_s, ps_y, ps_d, ps_p, W, Bf,
                     qT, flb, ones_b, dbg)


def _stage_a(nc, tc, g, sb, psA, psA2, W, Bp, Bf, MW, pool_ind_t, hs,
             ones2, ones1, qT, flb, dbg):
    xs = g["x_strip"]
    # contrib k/v layout: [y, x64, t, c] -- frame INSIDE so stage-B patch
    # gathers merge (x_slice, t, c) into one contiguous dim (3-dim DMA cap)
    kc = g["contrib"][0, OFF_K:OFF_V].rearrange("(y x t c) -> y x t c",
                                                y=STRIP_H, x=64, t=T)
    vc = g["contrib"][0, OFF_V:OFF_PK].rearrange("(y x t c) -> y x t c",
                                                 y=STRIP_H, x=64, t=T)

    # ---- x^T via XBAR (frames 0,1 and 2,3; full 10 strip rows) ----
    xt01 = sb.tile([128, NC4, PIX01], dt.bfloat16)
    nc.sync.dma_start_transpose(
        xt01, xs[0:2].rearrange("t y x c -> (t y x) c"))
    xt23 = sb.tile([128, NC4, PIX01], dt.bfloat16)
    nc.sync.dma_start_transpose(
        xt23, xs[2:4].rearrange("t y x c -> (t y x) c"))

    maskb = sb.tile([128, PIX01], dt.bfloat16)
    nc.sync.dma_start(maskb, g["mask_strip"][:, :, :]
                      .rearrange("l y x -> (l y x)")
                      .unsqueeze(0).to_broadcast([128, PIX01]))

    # ---- f01 k/v projections (transposed) + bias + mask -> mod inputs ----
    kmod_in = sb.tile([128, NC4, PIX01], dt.bfloat16)
    vmod_in = sb.tile([128, NC4, PIX01], dt.bfloat16)
    for dst, wt, bp in ((kmod_in, W["wk"], Bp["bk"]), (vmod_in, W["wv"], Bp["bv"])):
        for oc in range(NC4):
            for f in range(2):
                ps = psA.tile([128, DIM], dt.float32, tag="a512")
                for ic in range(NC4):
                    nc.tensor.matmul(ps[:, 0:480], wt[:, ic, oc * 128:(oc + 1) * 128],
                                     xt01[:, ic, f * 480:(f + 1) * 480],
                                     start=(ic == 0), stop=(ic == NC4 - 1))
                nc.vector.scalar_tensor_tensor(
                    dst[:, oc, f * 480:(f + 1) * 480], ps[:, 0:480],
                    bp[:, oc:oc + 1], maskb[:, f * 480:(f + 1) * 480],
                    ALU.add, ALU.add)

    # ---- f23 k/v natural projections (rows 1..9 of each frame) ----
    k23s = sb.tile([96, 2, 4, DIM], dt.bfloat16)
    v23s = sb.tile([96, 2, 4, DIM], dt.bfloat16)
    for dst, wt, bf in ((k23s, W["wk"], Bf["bk"]), (v23s, W["wv"], Bf["bv"])):
        for f in range(2):
            for grp in range(4):
                s = f * 480 + WI + grp * 96
                ps = psA.tile([128, DIM], dt.float32, tag="a512")
                for ic in range(NC4):
                    nc.tensor.matmul(ps[0:96, :], xt23[:, ic, s:s + 96],
                                     wt[:, ic, :], start=(ic == 0),
                                     stop=(ic == NC4 - 1))
                nc.vector.tensor_add(dst[:, f, grp, :], ps[0:96, :],
                                     bf[0:96, :])
    for dst_d, srct in ((kc, k23s), (vc, v23s)):
        for f in range(2):
            for y2 in range(2):
                nc.sync.dma_start(
                    dst_d[y2::2, 0:48, 2 + f, :]
                    .rearrange("y x c -> x y c"),
                    srct[y2 * 48:(y2 + 1) * 48, f, :, :])

    # ---- modulators on f01 (transposed domain) ----
    k01nat = _modulator(nc, tc, sb, psA, psA2, MW["k"], kmod_in, hs, "k")
    v01nat = _modulator(nc, tc, sb, psA, psA2, MW["v"], vmod_in, hs, "v")

    # contrib writes f01 (64-pitch pixel layout, one DMA per frame)
    for dst_d, srct in ((kc, k01nat), (vc, v01nat)):
        for f in range(2):
            nc.sync.dma_start(
                dst_d[:, :, f, :].rearrange("y x c -> (y x) c")
                .rearrange("(d p) c -> p d c", p=128),
                srct[:, 4 * f:4 * f + 4, :, :]
                .rearrange("p d a c -> p d (a c)"))

    # ---- x-margin duplication (cols 48:64 <- cols 0:16), DRAM->DRAM ----
    nc.sync.dma_start(kc[:, 48:64, :, :], kc[:, 0:16, :, :])
    nc.sync.dma_start(vc[:, 48:64, :, :], vc[:, 0:16, :, :])
    if "k" in dbg:
        nc.sync.dma_start(
            dbg["k"][:, :, :, :].rearrange("y x t c -> (y x t c)"),
            g["contrib"][0, OFF_K:OFF_V])
        nc.sync.dma_start(
            dbg["v"][:, :, :, :].rearrange("y x t c -> (y x t c)"),
            g["contrib"][0, OFF_V:OFF_PK])

    # ---- pooled tokens: pxT directly via indicator matmul ----
    xp = sb.tile([128, 9, DIM], dt.bfloat16)
    nc.sync.dma_start(xp, g["x_pool"][:, :, :].rearrange("y x c -> (y x) c")
                      .rearrange("(n p) c -> p n c", p=128))
    psx = psA.tile([128, DIM], dt.float32, tag="a512")
    for oc in range(NC4):
        for n in range(9):
            nc.tensor.matmul(psx[:, oc * 128:oc * 128 + POOL_PC],
                             xp[:, n, oc * 128:(oc + 1) * 128],
                             pool_ind_t[:, n, :], start=(n == 0), stop=(n == 8))
    pxT = sb.tile([128, NC4, POOL_PC], dt.bfloat16)
    for oc in range(NC4):
        nc.scalar.activation(pxT[:, oc, :],
                             psx[:, oc * 128:oc * 128 + POOL_PC],
                             AF.Identity, bias=Bp["pool_b"][:, oc:oc + 1])
    # pk^T [128, 4oc, 72]
    pkt = sb.tile([128, NC4, POOL_PC], dt.bfloat16)
    for oc in range(NC4):
        ps = psA.tile([128, DIM], dt.float32, tag="a512")
        for ic in range(NC4):
            nc.tensor.matmul(ps[:, 0:POOL_PC], W["wk"][:, ic, oc * 128:(oc + 1) * 128],
                             pxT[:, ic, :], start=(ic == 0), stop=(ic == NC4 - 1))
        nc.scalar.activation(pkt[:, oc, :], ps[:, 0:POOL_PC], AF.Identity,
                             bias=Bp["bk"][:, oc:oc + 1])
    nc.sync.dma_start(
        g["contrib"][0, OFF_PK:OFF_PV].rearrange("(p a n) -> p a n", p=128, a=NC4),
        pkt)
    # pv natural [72, 512]
    psv = psA.tile([128, DIM], dt.float32, tag="a512")
    for ic in range(NC4):
        nc.tensor.matmul(psv[0:POOL_PC, :], pxT[:, ic, :], W["wv"][:, ic, :],
                         start=(ic == 0), stop=(ic == NC4 - 1))
    pvt = sb.tile([POOL_PC, DIM], dt.bfloat16)
    nc.vector.tensor_add(pvt, psv[0:POOL_PC, :], Bf["bv"][0:POOL_PC, :])
    nc.sync.dma_start(
        g["contrib"][0, OFF_PV:AG_TOT].rearrange("(n c) -> n c", n=POOL_PC), pvt)
    if "pk" in dbg:
        nc.sync.dma_start(dbg["pk"][:, :, :].rearrange("p a n -> (p a n)"),
                          g["contrib"][0, OFF_PK:OFF_PV])
        nc.sync.dma_start(dbg["pv"][:, :].rearrange("n c -> (n c)"),
                          g["contrib"][0, OFF_PV:AG_TOT])

    # ---- q^T for this core's 5 windows ----
    xtw = sb.tile([128, NC4, WPC * NQ], dt.bfloat16)
    nc.sync.dma_start_transpose(
        xtw, g["x_win"][:, :, :, :].rearrange("w t p c -> (w t p) c"))
    for oc in range(NC4):
        for piece in range(4):
            s = piece * 320
            ps = psA.tile([128, DIM], dt.float32, tag="a512")
            for ic in range(NC4):
                nc.tensor.matmul(ps[:, 0:320], W["wq"][:, ic, oc * 128:(oc + 1) * 128],
                                 xtw[:, ic, s:s + 320],
                                 start=(ic == 0), stop=(ic == NC4 - 1))
            nc.scalar.activation(qT[:, oc, s:s + 320], ps[:, 0:320], AF.Identity,
                                 bias=Bp["bq"][:, oc:oc + 1])
    if "q" in dbg:
        nc.sync.dma_start(dbg["q"][:, :, :], qT)

    # ---- per-window mask flags -> flb [128, 5] ----
    mwt = sb.tile([L_T, WPC * WN], dt.float32)
    nc.sync.dma_start(mwt, g["mask_win"][:, :, :].rearrange("l w p -> l (w p)"))
    mx = sb.tile([L_T, WPC, 1], dt.float32)
    nc.vector.reduce_max(mx, mwt.rearrange("l (w p) -> l w p", w=WPC),
                         axis=AX.X, opt_input=False, opt_output=False)
    psf = psA.tile([128, DIM], dt.float32, tag="a512")
    nc.tensor.matmul(psf[0:1, 0:WPC], ones2, mx[:, :, 0], start=True, stop=True)
    fl = sb.tile([1, WPC], dt.float32)
    nc.scalar.activation(fl, psf[0:1, 0:WPC], AF.Sign)
    psf2 = psA.tile([128, DIM], dt.float32, tag="a512")
    nc.tensor.matmul(psf2[:, 0:WPC], ones1, fl, start=True, stop=True)
    nc.vector.tensor_copy(flb, psf2[:, 0:WPC])
    if "flags" in dbg:
        nc.sync.dma_start(dbg["flags"][:, :], flb)


def _modulator(nc, tc, sb, psA, psA2, mw, mod_in, hs, tag):
    """Modulator on transposed f01 [128, 4, 960] bf16. Returns natural
    64-pitch pixel tile [128, 8, 4, 128] (pix = f*512+y*64+x, rows 1..9)."""
    outT = sb.tile([128, NC4, 2, STRIP_H, 64], dt.bfloat16, tag=f"modo{tag}",
                   name=f"modo{tag}")
    nc.vector.memset(outT[:, :, :, :, 48:64], 0.0)
    with tc.tile_pool(name=f"sbM{tag}", bufs=1) as sbm:
        # conv1 1x1 512->128 + bias + leaky relu -> padded [128, 2, 10, 50]
        lx1 = sbm.tile([128, 2, HP, 50], dt.bfloat16, tag="lx1")
        nc.vector.memset(lx1[:, :, :, 0:1], 0.0)
        nc.vector.memset(lx1[:, :, :, 49:50], 0.0)
        for f in range(2):
            ps = psA2.tile([128, HP * WI], dt.float32, tag="mid")
            for ic in range(NC4):
                nc.tensor.matmul(ps, mw["sq"][:, ic, :],
                                 mod_in[:, ic, f * 480:(f + 1) * 480],
                                 start=(ic == 0), stop=(ic == NC4 - 1))
            nc.scalar.activation(lx1[:, f, :, 1:49],
                                 ps.rearrange("p (y x) -> p y x", y=HP),
                                 AF.Lrelu, bias=mw["sqb"][:, 0:1], alpha=0.2)
        # zero the halo rows at image edges
        for f in range(2):
            nc.vector.tensor_scalar_mul(lx1[:, f, 0, :], lx1[:, f, 0, :],
                                        hs[:, 0:1])
            nc.vector.tensor_scalar_mul(lx1[:, f, HP - 1, :],
                                        lx1[:, f, HP - 1, :], hs[:, 1:2])
        # conv2 3x3 128->128 + bias + SiLU
        lx2 = sbm.tile([128, 2, STRIP_H * WI], dt.bfloat16, tag="lx2")
        for f in range(2):
            ps = psA2.tile([128, STRIP_H * WI], dt.float32, tag="mid")
            ti = 0
            for dy in (-1, 0, 1):
                for dx in (-1, 0, 1):
                    rhs = lx1[:, f, 1 + dy:1 + dy + STRIP_H, 1 + dx:1 + dx + WI]
                    nc.tensor.matmul(ps, mw["f"][:, ti, :], rhs,
                                     start=(ti == 0), stop=(ti == 8))
                    ti += 1
            nc.scalar.activation(lx2[:, f, :], ps, AF.Silu, bias=mw["fb"][:, 0:1])
        # conv3 1x1 128->512 + bias -> transposed 64-pitch tile
        for f in range(2):
            for oc in range(NC4):
                ps = psA2.tile([128, STRIP_H * WI], dt.float32, tag="mid")
                nc.tensor.matmul(ps, mw["un"][:, oc * 128:(oc + 1) * 128],
                                 lx2[:, f, :], start=True, stop=True)
                nc.scalar.activation(outT[:, oc, f, :, 0:48],
                                     ps.rearrange("p (y x) -> p y x", y=STRIP_H),
                                     AF.Identity, bias=mw["unb"][:, oc:oc + 1])
    # back to natural pixel-major via XBAR: [128pix, 8 chunks, 4oc, 128]
    nat = sb.tile([128, 8, NC4, 128], dt.bfloat16, tag=f"nat{tag}",
                  name=f"nat{tag}")
    for oc in range(NC4):
        nc.sync.dma_start_transpose(
            nat[:, :, oc, :],
            outT[:, oc, :, :, :].rearrange("p f y x -> p (f y x)"))
    return nat


def _stage_b(nc, tc, g, sb, sb1, ps_s, ps_y, ps_d, ps_p, W, Bf, qT, flb,
             ones_b, dbg):
    ag = g["agout"]
    kgv = ag[:, OFF_K:OFF_V].rearrange("s (y x t c) -> s y x t c",
                                       y=STRIP_H, x=64, t=T)
    vgv = ag[:, OFF_V:OFF_PK].rearrange("s (y x t c) -> s y x t c",
                                        y=STRIP_H, x=64, t=T)

    # pooled keys resident
    pk = sb1.tile([128, NC4, N_CORES * POOL_PC], dt.bfloat16)
    for s in range(N_CORES):
        nc.sync.dma_start(
            pk[:, :, s * POOL_PC:(s + 1) * POOL_PC],
            ag[s, OFF_PK:OFF_PV].rearrange("(p a n) -> p a n", p=128, a=NC4))
    pv = sb1.tile([128, NPCH, DIM], dt.bfloat16)
    row = 0
    for s in range(N_CORES):
        r = 0
        while r < POOL_PC:
            ch, off = (row + r) // 128, (row + r) % 128
            n = min(POOL_PC - r, 128 - off)
            nc.sync.dma_start(
                pv[off:off + n, ch, :],
                ag[s, OFF_PV + r * DIM:OFF_PV + (r + n) * DIM]
                .rearrange("(n c) -> n c", c=DIM))
            r += n
        row += POOL_PC

    # dynamic-offset registers
    meta_t = sb1.tile([1, META_LEN], dt.int32)
    nc.sync.dma_start(meta_t, g["meta"][:].unsqueeze(0))
    regs = []
    for i in range(WPC * REGS_PER_WIN):
        r = nc.alloc_register(mybir.EngineType.SP, f"mreg{i}")
        nc.sync.reg_load(r, meta_t[0:1, i:i + 1])
        regs.append(nc.sync.snap(r))

    ds = bass.ds
    for wi in range(WPC):
        r_prev, r_cur, r_next, r_x0, r_xo, r_x12 = \
            regs[wi * REGS_PER_WIN:(wi + 1) * REGS_PER_WIN]

        # ---- patch gathers (natural layout, own-pixels-first) ----
        kp = sb.tile([128, 8, DIM], dt.bfloat16, tag="kp")
        vp = sb.tile([128, 8, DIM], dt.bfloat16, tag="vp")
        for dst, src in ((kp, kgv), (vp, vgv)):
            # own window: chunk 2f partitions 0:64
            nc.sync.dma_start(
                dst[0:64, 0:8:2, :],
                src[ds(r_cur, 1), :, ds(r_xo, 8), :, :].squeeze(0))
            # top margin rows (prev strip rows 4:8): chunk 2f partitions 64:128
            nc.sync.dma_start(
                dst[64:128, 0:8:2, :],
                src[ds(r_prev, 1), 4:8, ds(r_x0, 16), :, :].squeeze(0))
            # bottom margin rows (next strip rows 0:4): chunk 2f+1 p 0:64
            nc.sync.dma_start(
                dst[0:64, 1:8:2, :],
                src[ds(r_next, 1), 0:4, ds(r_x0, 16), :, :].squeeze(0))
            # left margin cols: chunk 2f+1 p 64:96
            nc.sync.dma_start(
                dst[64:96, 1:8:2, :],
                src[ds(r_cur, 1), :, ds(r_x0, 4), :, :].squeeze(0))
            # right margin cols: chunk 2f+1 p 96:128
            nc.sync.dma_start(
                dst[96:128, 1:8:2, :],
                src[ds(r_cur, 1), :, ds(r_x12, 4), :, :].squeeze(0))
        # k patch transpose: [128ch, (slot, cchunk), 128pix]
        kpT = sb.tile([128, 32, 128], dt.bfloat16, tag="kpT")
        nc.sync.dma_start_transpose(
            kpT, kp[:, :, :].rearrange("p s c -> p (s c)"))
        if wi == 0 and "kp" in dbg:
            nc.sync.dma_start(dbg["kp"][:, :, :], kp)
            nc.sync.dma_start(dbg["vp"][:, :, :], vp)
            nc.sync.dma_start(dbg["kpT"][:, :, :], kpT)

        q_w = qT[:, :, wi * NQ:(wi + 1) * NQ]  # [128, 4h, 256]
        yfin = [sb.tile([128, DIM], dt.bfloat16, tag=f"yfin{h2}", name=f"yfin{h2}")
                for h2 in range(2)]

        for h in range(N_HEAD):
            # ---- scores + exp (groups of <=4 chunks) ----
            pT = sb.tile([128, NCHUNK, NQ], dt.bfloat16, tag="pT", name="pT")
            for g0, g1 in EXP_GROUPS:
                ps = ps_s.tile([128, 4, NQ], dt.float32, tag="s")
                for j in range(g0, g1):
                    if j < 8:
                        lhs = kpT[:, j * 4 + h, :]
                        n = 128
                    else:
                        c0 = (j - 8) * 128
                        n = min(128, NPOOL - c0)
                        lhs = pk[:, h, c0:c0 + n]
                    nc.tensor.matmul(ps[0:n, j - g0, :], lhs, q_w[:, h, :],
                                     start=True, stop=True)
                nc.scalar.activation(pT[:, g0:g1, :], ps[:, 0:g1 - g0, :],
                                     AF.Exp, scale=SCALE)
            if wi == 0 and h == 0 and "pT" in dbg:
                nc.sync.dma_start(dbg["pT"][:, :, :], pT)

            # ---- denominators (tiny ap=1 matmuls) ----
            dn = ps_d.tile([128, 8], dt.float32, tag="d")
            for half in range(2):
                for j in range(NCHUNK):
                    n = 128 if j < 12 else 64
                    nc.tensor.matmul(dn[:, half:half + 1],
                                     pT[0:n, j, half * 128:(half + 1) * 128],
                                     ones_b[0:n, :],
                                     start=(j == 0), stop=(j == NCHUNK - 1))
            for f in range(T):
                nc.tensor.matmul(
                    dn[(f % 2) * 64:(f % 2) * 64 + 64, 2 + f // 2:3 + f // 2],
                    pT[0:64, 2 * f, f * WN:(f + 1) * WN], ones_b[0:64, :],
                    start=True, stop=True)

            # ---- AV (transposed): yT [q, ch] ----
            ys = ps_y.tile([128, 4, 128], dt.float32, tag="y")
            for half in range(2):
                for j in range(NCHUNK):
                    n = 128 if j < 12 else 64
                    rhs = (vp[0:n, j, h * 128:(h + 1) * 128] if j < 8
                           else pv[0:n, j - 8, h * 128:(h + 1) * 128])
                    nc.tensor.matmul(ys[:, half, :],
                                     pT[0:n, j, half * 128:(half + 1) * 128],
                                     rhs, start=(j == 0), stop=(j == NCHUNK - 1))
            for f in range(T):
                nc.tensor.matmul(
                    ys[(f % 2) * 64:(f % 2) * 64 + 64, 2 + f // 2, :],
                    pT[0:64, 2 * f, f * WN:(f + 1) * WN],
                    vp[0:64, 2 * f, h * 128:(h + 1) * 128],
                    start=True, stop=True)

            # ---- normalize + blend ----
            rec = sb.tile([128, 4], dt.float32, tag="rec", name="rec")
            nc.vector.reciprocal(rec, dn[:, 0:4])
            for half in range(2):
                yln = sb.tile([128, 128], dt.bfloat16, tag="yln", name="yln")
                nc.vector.tensor_scalar_mul(yln, ys[:, 2 + half, :],
                                            rec[:, 2 + half:3 + half])
                dlt = sb.tile([128, 128], dt.bfloat16, tag="dlt", name="dlt")
                nc.vector.scalar_tensor_tensor(
                    dlt, ys[:, half, :], rec[:, half:half + 1], yln,
                    ALU.mult, ALU.subtract)
                nc.vector.scalar_tensor_tensor(
                    yfin[half][:, h * 128:(h + 1) * 128], dlt,
                    flb[:, wi:wi + 1], yln, ALU.mult, ALU.add)

        # ---- output projection ----
        for half in range(2):
            yTT = sb.tile([128, NC4, 128], dt.bfloat16, tag="yTT", name="yTT")
            nc.sync.dma_start_transpose(yTT, yfin[half][:, :])
            pso = ps_p.tile([128, DIM], dt.float32, tag="proj")
            for h in range(N_HEAD):
                nc.tensor.matmul(pso, yTT[:, h, :], W["wp"][:, h, :],
                                 start=(h == 0), stop=(h == N_HEAD - 1))
            osb = sb.tile([128, DIM], dt.float32, tag="osb", name="osb")
            nc.vector.tensor_add(osb, pso, Bf["bp"])
            nc.sync.dma_start(
                g["out_win"][wi, 2 * half:2 * half + 2, :, :]
                .rearrange("t p c -> (t p) c"), osb)


# ==================== host side ====================

def _host_inputs(inputs, debug=False):
    x = np.asarray(inputs["x"], np.float32)[0]  # [4,48,48,512]
    mask = np.asarray(inputs["mask"], np.float32)[0, :, :, :, 0]  # [2,48,48]
    xb = x.astype(BF)
    maskb = mask.astype(BF)

    pool_ind = np.zeros((128, 9, POOL_PC), BF)
    for n in range(9):
        for p in range(128):
            pix = n * 128 + p
            y, xx = pix // WI, pix % WI
            pool_ind[p, n, (y // PH) * PGRID + xx // PW] = BF(1.0 / (PH * PW))

    common = dict(
        wq=np.asarray(inputs["Wq"], np.float32).astype(BF),
        wk=np.asarray(inputs["Wk"], np.float32).astype(BF),
        wv=np.asarray(inputs["Wv"], np.float32).astype(BF),
        wp=np.asarray(inputs["Wp"], np.float32).astype(BF),
        bq=np.asarray(inputs["bq"], np.float32),
        bk=np.asarray(inputs["bk"], np.float32),
        bv=np.asarray(inputs["bv"], np.float32),
        bp=np.asarray(inputs["bp"], np.float32),
        pool_b=np.asarray(inputs["pool_b"], np.float32),
        pool_ind=pool_ind,
    )
    for tag, pre in (("k", "kmod"), ("v", "vmod")):
        common[f"{tag}sq"] = np.ascontiguousarray(
            np.asarray(inputs[f"{pre}_sq_w"], np.float32)[:, :, 0, 0].T).astype(BF)
        common[f"{tag}sqb"] = np.asarray(inputs[f"{pre}_sq_b"], np.float32)
        fw = np.asarray(inputs[f"{pre}_f_w"], np.float32)
        common[f"{tag}f"] = np.ascontiguousarray(
            np.stack([fw[:, :, dy, dx].T for dy in range(3) for dx in range(3)])
        ).astype(BF)
        common[f"{tag}fb"] = np.asarray(inputs[f"{pre}_f_b"], np.float32)
        common[f"{tag}un"] = np.ascontiguousarray(
            np.asarray(inputs[f"{pre}_un_w"], np.float32)[:, :, 0, 0].T).astype(BF)
        common[f"{tag}unb"] = np.asarray(inputs[f"{pre}_un_b"], np.float32)

    in_maps = []
    for c in range(N_CORES):
        m = dict(common)
        if c < N_STRIP:
            r0 = c * STRIP_H
            xs = np.zeros((T, HP, WI, DIM), BF)
            ms = np.zeros((L_T, HP, WI), BF)
            lo, hi = max(0, r0 - 1), min(HI, r0 + STRIP_H + 1)
            xs[:, lo - (r0 - 1):lo - (r0 - 1) + hi - lo] = xb[:, lo:hi]
            ms[:, lo - (r0 - 1):lo - (r0 - 1) + hi - lo] = maskb[:, lo:hi]
            m["x_strip"] = xs
            m["mask_strip"] = ms
            m["halo_scale"] = np.array(
                [0.0 if r0 == 0 else 1.0,
                 0.0 if r0 + STRIP_H == HI else 1.0], np.float32)
        else:
            m["x_strip"] = np.zeros((T, HP, WI, DIM), BF)
            m["mask_strip"] = np.zeros((L_T, HP, WI), BF)
            m["halo_scale"] = np.ones(2, np.float32)
        xw = np.zeros((WPC, T, WN, DIM), BF)
        mw = np.zeros((L_T, WPC, WN), np.float32)
        for k, w in enumerate(WIN_ASSIGN[c]):
            i, j = w // N_WW, w % N_WW
            blk = xb[:, 8 * i:8 * i + 8, 8 * j:8 * j + 8, :]
            xw[k] = blk.reshape(T, WN, DIM)
            mw[:, k] = mask[:, 8 * i:8 * i + 8, 8 * j:8 * j + 8].reshape(L_T, WN)
        m["x_win"] = xw
        m["mask_win"] = mw
        f, pr0 = c // 2, 6 * (c % 2)
        m["x_pool"] = np.ascontiguousarray(xb[f, pr0 * 4:pr0 * 4 + 24])
        m["meta"] = _meta_for_core(c)
        in_maps.append(m)
    return in_maps


def _get_nc(debug=False):
    key = bool(debug)
    if key not in _NC_CACHE:
        _NC_CACHE[key] = build_nc(debug=debug)
    return _NC_CACHE[key]


def run_spmd(inputs, debug=False):
    nc = _get_nc(debug=debug)
    in_maps = _host_inputs(inputs, debug=debug)
    res = run_bass_kernel_spmd(nc, in_maps, list(range(N_CORES)))
    return res


def assemble(results):
    out = np.zeros((T, HI, WI, DIM), np.float32)
    done = set()
    for c in range(N_CORES):
        ow = results[c]["out_win"]
        for k, w in enumerate(WIN_ASSIGN[c]):
            if w in done:
                continue
            done.add(w)
            i, j = w // N_WW, w % N_WW
            out[:, 8 * i:8 * i + 8, 8 * j:8 * j + 8, :] = \
                ow[k].reshape(T, WH, WW, DIM)
    return out[None]


def kernel(**inputs):
    res = run_spmd(inputs)
    return assemble(res.results)


# revision 29
# speedup vs baseline: 2.9721x; 1.0383x over previous
"""Trainium2 Bass kernel for nn_ErrorAwareSelfAttention (8 NeuronCores).

v2 design (bf16 matmul pipeline):
- All heavy matmuls run in bf16 (1 cyc/row vs fp32's 4) with fp32 PSUM
  accumulation; host pre-converts x / weights to bf16.
- Stage A (strips of 8 image rows on cores 0-5, SPMD on all):
  x^T via XBAR dma-transpose straight from DRAM; k/v projections;
  Modulator convs on frames 0,1 (Lrelu/Silu on ACT); k and v both
  written to DRAM contribs in NATURAL [t, y, x64, ch] layout (ch
  contiguous -> 1KB DMA runs); pooled tokens (pk^T, pv) sharded over
  all 8 cores; one bf16 AllGather publishes everything.
- Stage B (5 windows per core): the 16x16 halo patch of k and v is
  gathered in natural layout (5 DMAs each, own-pixels-first ordering),
  k patch transposed on-chip by one XBAR DMA; scores per head in 13
  key chunks (8 patch + 5 pooled); exp on ACT in groups of 4 chunks
  (PSUM-bank sized); transposed AV produces y^T [q, ch] so softmax
  denominators are per-PARTITION scalars: denoms via tiny ap=1
  ones-matmuls on PE, normalization via DVE tensor_scalar; the local
  (unmasked) path reuses the global exp values (own-window keys are
  partitions 0:64 of even chunks) - no second exp; blend by per-window
  mask flag; y^T transposed back by XBAR for the output projection.
"""

import math
import sys

sys.path.insert(0, "/opt/trn_rl_repo")

import numpy as np
import ml_dtypes

import concourse.bass as bass
import concourse.mybir as mybir
import concourse.tile as tile
from concourse import bacc
from concourse.bass_utils import run_bass_kernel_spmd

dt = mybir.dt
AF = mybir.ActivationFunctionType
AX = mybir.AxisListType
ALU = mybir.AluOpType
BF = ml_dtypes.bfloat16

# ---------------- problem constants (hardcoded) ----------------
DIM = 512
N_HEAD = 4
CH = 128
WH = WW = 8
EH = EW = 4
PH = PW = 4
B, T, HI, WI = 1, 4, 48, 48
L_T = 2
N_WH = N_WW = 6
NW = 36
WN = 64
SCALE = 1.0 / math.sqrt(CH)
N_CORES = 8
NC4 = 4  # 512 / 128 channel chunks

STRIP_H = 8
N_STRIP = 6
HP = STRIP_H + 2          # strip rows incl 1-row halo
PIX01 = 2 * HP * WI       # 960
WPC = 5
_bounds = [int(NW * c / N_CORES) for c in range(N_CORES + 1)]
WIN_ASSIGN = []
for c in range(N_CORES):
    ws = list(range(_bounds[c], _bounds[c + 1]))
    while len(ws) < WPC:
        ws.append(ws[-1])
    WIN_ASSIGN.append(ws)

PGRID = HI // PH          # 12
NPOOL = T * PGRID * PGRID  # 576
POOL_PC = NPOOL // N_CORES  # 72
NPCH = 5                  # pool key chunks: 4x128 + 64
NQ = T * WN               # 256 queries / window
NPATCH = 8                # patch chunks (128 keys each)
NCHUNK = NPATCH + NPCH    # total key chunks per window

# AllGather packing (bf16 elements)
OFF_K = 0
OFF_V = OFF_K + T * STRIP_H * 64 * DIM
OFF_PK = OFF_V + T * STRIP_H * 64 * DIM
OFF_PV = OFF_PK + 128 * NC4 * POOL_PC
AG_TOT = OFF_PV + POOL_PC * DIM
AG_OUT_BYTES = N_CORES * AG_TOT * 2

REGS_PER_WIN = 6  # prev, cur, next, x0, xo, x12

_NC_CACHE = {}


def _meta_for_core(c):
    vals = []
    for w in WIN_ASSIGN[c]:
        i, j = w // N_WW, w % N_WW
        x0 = (8 * j - 4) % 48
        vals += [(i - 1) % N_STRIP, i, (i + 1) % N_STRIP, x0, 8 * j, x0 + 12]
    pad = (-len(vals)) % 32
    vals += [0] * pad
    return np.asarray(vals, np.int32)


META_LEN = len(_meta_for_core(0))


def build_nc(debug=False):
    nc = bacc.Bacc("TRN2", target_bir_lowering=False, debug=True)

    # ---------------- I/O ----------------
    x_strip = nc.dram_tensor("x_strip", [T, HP, WI, DIM], dt.bfloat16,
                             kind="ExternalInput")
    x_win = nc.dram_tensor("x_win", [WPC, T, WN, DIM], dt.bfloat16,
                           kind="ExternalInput")
    x_pool = nc.dram_tensor("x_pool", [24, WI, DIM], dt.bfloat16,
                            kind="ExternalInput")
    mask_strip = nc.dram_tensor("mask_strip", [L_T, HP, WI], dt.bfloat16,
                                kind="ExternalInput")
    mask_win = nc.dram_tensor("mask_win", [L_T, WPC, WN], dt.float32,
                              kind="ExternalInput")
    halo_scale = nc.dram_tensor("halo_scale", [2], dt.float32,
                                kind="ExternalInput")
    meta = nc.dram_tensor("meta", [META_LEN], dt.int32, kind="ExternalInput")
    pool_ind = nc.dram_tensor("pool_ind", [128, 9, POOL_PC], dt.bfloat16,
                              kind="ExternalInput")

    wtens = {}
    for nm in ("wq", "wk", "wv", "wp"):
        wtens[nm] = nc.dram_tensor(nm, [DIM, DIM], dt.bfloat16,
                                   kind="ExternalInput")
    for nm in ("bq", "bk", "bv", "bp", "pool_b"):
        wtens[nm] = nc.dram_tensor(nm, [DIM], dt.float32,
                                   kind="ExternalInput")
    mods = {}
    for tag in ("k", "v"):
        mods[tag] = dict(
            sq=nc.dram_tensor(f"{tag}sq", [DIM, 128], dt.bfloat16, kind="ExternalInput"),
            sqb=nc.dram_tensor(f"{tag}sqb", [128], dt.float32, kind="ExternalInput"),
            f=nc.dram_tensor(f"{tag}f", [9, 128, 128], dt.bfloat16, kind="ExternalInput"),
            fb=nc.dram_tensor(f"{tag}fb", [128], dt.float32, kind="ExternalInput"),
            un=nc.dram_tensor(f"{tag}un", [128, DIM], dt.bfloat16, kind="ExternalInput"),
            unb=nc.dram_tensor(f"{tag}unb", [DIM], dt.float32, kind="ExternalInput"),
        )

    out_win = nc.dram_tensor("out_win", [WPC, T, WN, DIM], dt.float32,
                             kind="ExternalOutput")
    dbg = {}
    if debug:
        dbg["k"] = nc.dram_tensor("dbg_k", [STRIP_H, 64, T, DIM], dt.bfloat16,
                                  kind="ExternalOutput")
        dbg["v"] = nc.dram_tensor("dbg_v", [STRIP_H, 64, T, DIM], dt.bfloat16,
                                  kind="ExternalOutput")
        dbg["pk"] = nc.dram_tensor("dbg_pk", [128, NC4, POOL_PC], dt.bfloat16,
                                   kind="ExternalOutput")
        dbg["pv"] = nc.dram_tensor("dbg_pv", [POOL_PC, DIM], dt.bfloat16,
                                   kind="ExternalOutput")
        dbg["q"] = nc.dram_tensor("dbg_q", [128, NC4, WPC * NQ], dt.bfloat16,
                                  kind="ExternalOutput")
        dbg["flags"] = nc.dram_tensor("dbg_flags", [128, WPC], dt.float32,
                                      kind="ExternalOutput")
        dbg["kp"] = nc.dram_tensor("dbg_kp", [128, 8, DIM], dt.bfloat16,
                                   kind="ExternalOutput")
        dbg["vp"] = nc.dram_tensor("dbg_vp", [128, 8, DIM], dt.bfloat16,
                                   kind="ExternalOutput")
        dbg["kpT"] = nc.dram_tensor("dbg_kpT", [128, 32, 128], dt.bfloat16,
                                    kind="ExternalOutput")
        dbg["pT"] = nc.dram_tensor("dbg_pT", [128, NCHUNK, NQ], dt.bfloat16,
                                   kind="ExternalOutput")

    contrib = nc.dram_tensor("contrib", [1, AG_TOT], dt.bfloat16)
    agout = nc.dram_tensor("agout", [N_CORES, AG_TOT], dt.bfloat16,
                           addr_space="Shared")

    g = dict(locals())
    g.update(wtens)
    with tile.TileContext(nc, num_cores=N_CORES) as tc:
        _program(nc, tc, g)
    nc.compile()
    return nc


def _program(nc, tc, g):
    dbg = g["dbg"]
    with (
        tc.tile_pool(name="wpool", bufs=1) as wpool,
        tc.tile_pool(name="persist", bufs=1) as sbP,
    ):
        # ---- weights / consts ----
        W = {}
        for nm in ("wq", "wk", "wv", "wp"):
            t = wpool.tile([128, NC4, DIM], dt.bfloat16, tag=nm, name=nm)
            nc.sync.dma_start(t, g[nm][:, :].rearrange("(a p) o -> p a o", p=128))
            W[nm] = t
        Bp = {}
        for nm in ("bq", "bk", "bv", "pool_b"):
            t = wpool.tile([128, NC4], dt.float32, tag=nm + "p", name=nm + "p")
            nc.sync.dma_start(t, g[nm][:].rearrange("(a p) -> p a", p=128))
            Bp[nm] = t
        Bf = {}
        for nm in ("bk", "bv", "bp", "pool_b"):
            t = wpool.tile([128, DIM], dt.float32, tag=nm + "f", name=nm + "f")
            nc.sync.dma_start(t, g[nm][:].unsqueeze(0).to_broadcast([128, DIM]))
            Bf[nm] = t
        MW = {}
        for tag in ("k", "v"):
            m = g["mods"][tag]
            MW[tag] = dict(
                sq=wpool.tile([128, NC4, 128], dt.bfloat16, tag=f"{tag}sqw", name=f"{tag}sqw"),
                sqb=wpool.tile([128, 1], dt.float32, tag=f"{tag}sqbw", name=f"{tag}sqbw"),
                f=wpool.tile([128, 9, 128], dt.bfloat16, tag=f"{tag}fw", name=f"{tag}fw"),
                fb=wpool.tile([128, 1], dt.float32, tag=f"{tag}fbw", name=f"{tag}fbw"),
                un=wpool.tile([128, DIM], dt.bfloat16, tag=f"{tag}unw", name=f"{tag}unw"),
                unb=wpool.tile([128, NC4], dt.float32, tag=f"{tag}unbw", name=f"{tag}unbw"),
            )
            nc.sync.dma_start(MW[tag]["sq"], m["sq"][:, :].rearrange("(a p) o -> p a o", p=128))
            nc.sync.dma_start(MW[tag]["sqb"], m["sqb"][:].unsqueeze(1))
            nc.sync.dma_start(MW[tag]["f"], m["f"][:, :, :].rearrange("n p o -> p n o"))
            nc.sync.dma_start(MW[tag]["fb"], m["fb"][:].unsqueeze(1))
            nc.sync.dma_start(MW[tag]["un"], m["un"][:, :])
            nc.sync.dma_start(MW[tag]["unb"], m["unb"][:].rearrange("(a p) -> p a", p=128))
        pool_ind_t = wpool.tile([128, 9, POOL_PC], dt.bfloat16)
        nc.sync.dma_start(pool_ind_t, g["pool_ind"][:, :, :])
        hs = wpool.tile([128, 2], dt.float32)
        nc.sync.dma_start(hs, g["halo_scale"][:].unsqueeze(0).to_broadcast([128, 2]))
        ones_b = wpool.tile([128, 1], dt.bfloat16)
        nc.vector.memset(ones_b, 1.0)
        ones2 = wpool.tile([2, 1], dt.float32)
        nc.vector.memset(ones2, 1.0)
        ones1 = wpool.tile([1, 128], dt.float32)
        nc.vector.memset(ones1, 1.0)

        # persistent across stages
        qT = sbP.tile([128, NC4, WPC * NQ], dt.bfloat16)
        flb = sbP.tile([128, WPC], dt.float32)

        # ================= stage A =================
        with (
            tc.tile_pool(name="psA", bufs=6, space="PSUM") as psA,
            tc.tile_pool(name="psA2", bufs=2, space="PSUM") as psA2,
            tc.tile_pool(name="sbA", bufs=1) as sbA,
        ):
            _stage_a(nc, tc, g, sbA, psA, psA2, W, Bp, Bf, MW, pool_ind_t,
                     hs, ones2, ones1, qT, flb, dbg)

        # ---------------- AllGather ----------------
        nc.gpsimd.collective_compute(
            "AllGather", mybir.AluOpType.bypass,
            ins=[g["contrib"][:, :]],
            outs=[g["agout"][:, :]],
            replica_groups=[list(range(N_CORES))],
        )

        # ================= stage B =================
        with (
            tc.tile_pool(name="sbB", bufs=3) as sbB,
            tc.tile_pool(name="sbB1", bufs=1) as sbB1,
        ):
            _stage_b(nc, tc, g, sbB, sbB1, W, Bf, qT, flb, ones_b, dbg)


def _stage_a(nc, tc, g, sb, psA, psA2, W, Bp, Bf, MW, pool_ind_t, hs,
             ones2, ones1, qT, flb, dbg):
    xs = g["x_strip"]
    # contrib k/v layout: [y, x64, t, c] -- frame INSIDE so stage-B patch
    # gathers merge (x_slice, t, c) into one contiguous dim (3-dim DMA cap)
    kc = g["contrib"][0, OFF_K:OFF_V].rearrange("(y x t c) -> y x t c",
                                                y=STRIP_H, x=64, t=T)
    vc = g["contrib"][0, OFF_V:OFF_PK].rearrange("(y x t c) -> y x t c",
                                                 y=STRIP_H, x=64, t=T)

    # ---- x^T via XBAR (frames 0,1 and 2,3; full 10 strip rows) ----
    xt01 = sb.tile([128, NC4, PIX01], dt.bfloat16)
    nc.sync.dma_start_transpose(
        xt01, xs[0:2].rearrange("t y x c -> (t y x) c"))
    xt23 = sb.tile([128, NC4, PIX01], dt.bfloat16)
    nc.sync.dma_start_transpose(
        xt23, xs[2:4].rearrange("t y x c -> (t y x) c"))

    maskb = sb.tile([128, PIX01], dt.bfloat16)
    nc.sync.dma_start(maskb, g["mask_strip"][:, :, :]
                      .rearrange("l y x -> (l y x)")
                      .unsqueeze(0).to_broadcast([128, PIX01]))

    # ---- f01 k/v projections (transposed) + bias + mask -> mod inputs ----
    kmod_in = sb.tile([128, NC4, PIX01], dt.bfloat16)
    vmod_in = sb.tile([128, NC4, PIX01], dt.bfloat16)
    for dst, wt, bp in ((kmod_in, W["wk"], Bp["bk"]), (vmod_in, W["wv"], Bp["bv"])):
        for oc in range(NC4):
            for f in range(2):
                ps = psA.tile([128, DIM], dt.float32, tag="a512")
                for ic in range(NC4):
                    nc.tensor.matmul(ps[:, 0:480], wt[:, ic, oc * 128:(oc + 1) * 128],
                                     xt01[:, ic, f * 480:(f + 1) * 480],
                                     start=(ic == 0), stop=(ic == NC4 - 1))
                nc.vector.scalar_tensor_tensor(
                    dst[:, oc, f * 480:(f + 1) * 480], ps[:, 0:480],
                    bp[:, oc:oc + 1], maskb[:, f * 480:(f + 1) * 480],
                    ALU.add, ALU.add)

    # ---- f23 k/v transposed projections -> 64-pitch tiles -> XBAR back ----
    k23T = sb.tile([128, NC4, 2, STRIP_H, 64], dt.bfloat16)
    v23T = sb.tile([128, NC4, 2, STRIP_H, 64], dt.bfloat16)
    nc.vector.memset(k23T[:, :, :, :, 48:64], 0.0)
    nc.vector.memset(v23T[:, :, :, :, 48:64], 0.0)
    for dstT, wt, bp in ((k23T, W["wk"], Bp["bk"]), (v23T, W["wv"], Bp["bv"])):
        for oc in range(NC4):
            for f in range(2):
                s = f * 480 + WI
                ps = psA.tile([128, DIM], dt.float32, tag="a512")
                for ic in range(NC4):
                    nc.tensor.matmul(ps[:, 0:384], wt[:, ic, oc * 128:(oc + 1) * 128],
                                     xt23[:, ic, s:s + 384],
                                     start=(ic == 0), stop=(ic == NC4 - 1))
                nc.scalar.activation(
                    dstT[:, oc, f, :, 0:48],
                    ps[:, 0:384].rearrange("p (y x) -> p y x", y=STRIP_H),
                    AF.Identity, bias=bp[:, oc:oc + 1])
    k23nat = sb.tile([128, 8, NC4, 128], dt.bfloat16)
    v23nat = sb.tile([128, 8, NC4, 128], dt.bfloat16)
    for nat, srcT in ((k23nat, k23T), (v23nat, v23T)):
        for oc in range(NC4):
            nc.sync.dma_start_transpose(
                nat[:, :, oc, :],
                srcT[:, oc, :, :, :].rearrange("p f y x -> p (f y x)"))
    for dst_d, srct in ((kc, k23nat), (vc, v23nat)):
        for f in range(2):
            nc.sync.dma_start(
                dst_d[:, :, 2 + f, :].rearrange("y x c -> (y x) c")
                .rearrange("(d p) c -> p d c", p=128),
                srct[:, 4 * f:4 * f + 4, :, :]
                .rearrange("p d a c -> p d (a c)"))

    # ---- modulators on f01 (transposed domain) ----
    k01nat = _modulator(nc, tc, sb, psA, psA2, MW["k"], kmod_in, hs, "k")
    v01nat = _modulator(nc, tc, sb, psA, psA2, MW["v"], vmod_in, hs, "v")

    # contrib writes f01 (64-pitch pixel layout, one DMA per frame)
    for dst_d, srct in ((kc, k01nat), (vc, v01nat)):
        for f in range(2):
            nc.sync.dma_start(
                dst_d[:, :, f, :].rearrange("y x c -> (y x) c")
                .rearrange("(d p) c -> p d c", p=128),
                srct[:, 4 * f:4 * f + 4, :, :]
                .rearrange("p d a c -> p d (a c)"))

    # ---- x-margin duplication (cols 48:64 <- cols 0:16), DRAM->DRAM ----
    nc.sync.dma_start(kc[:, 48:64, :, :], kc[:, 0:16, :, :])
    nc.sync.dma_start(vc[:, 48:64, :, :], vc[:, 0:16, :, :])
    if "k" in dbg:
        nc.sync.dma_start(
            dbg["k"][:, :, :, :].rearrange("y x t c -> (y x t c)"),
            g["contrib"][0, OFF_K:OFF_V])
        nc.sync.dma_start(
            dbg["v"][:, :, :, :].rearrange("y x t c -> (y x t c)"),
            g["contrib"][0, OFF_V:OFF_PK])

    # ---- pooled tokens: pxT directly via indicator matmul ----
    xp = sb.tile([128, 9, DIM], dt.bfloat16)
    nc.sync.dma_start(xp, g["x_pool"][:, :, :].rearrange("y x c -> (y x) c")
                      .rearrange("(n p) c -> p n c", p=128))
    psx = psA.tile([128, DIM], dt.float32, tag="a512")
    for oc in range(NC4):
        for n in range(9):
            nc.tensor.matmul(psx[:, oc * 128:oc * 128 + POOL_PC],
                             xp[:, n, oc * 128:(oc + 1) * 128],
                             pool_ind_t[:, n, :], start=(n == 0), stop=(n == 8))
    pxT = sb.tile([128, NC4, POOL_PC], dt.bfloat16)
    for oc in range(NC4):
        nc.scalar.activation(pxT[:, oc, :],
                             psx[:, oc * 128:oc * 128 + POOL_PC],
                             AF.Identity, bias=Bp["pool_b"][:, oc:oc + 1])
    # pk^T [128, 4oc, 72]
    pkt = sb.tile([128, NC4, POOL_PC], dt.bfloat16)
    for oc in range(NC4):
        ps = psA.tile([128, DIM], dt.float32, tag="a512")
        for ic in range(NC4):
            nc.tensor.matmul(ps[:, 0:POOL_PC], W["wk"][:, ic, oc * 128:(oc + 1) * 128],
                             pxT[:, ic, :], start=(ic == 0), stop=(ic == NC4 - 1))
        nc.scalar.activation(pkt[:, oc, :], ps[:, 0:POOL_PC], AF.Identity,
                             bias=Bp["bk"][:, oc:oc + 1])
    nc.sync.dma_start(
        g["contrib"][0, OFF_PK:OFF_PV].rearrange("(p a n) -> p a n", p=128, a=NC4),
        pkt)
    # pv natural [72, 512]
    psv = psA.tile([128, DIM], dt.float32, tag="a512")
    for ic in range(NC4):
        nc.tensor.matmul(psv[0:POOL_PC, :], pxT[:, ic, :], W["wv"][:, ic, :],
                         start=(ic == 0), stop=(ic == NC4 - 1))
    pvt = sb.tile([POOL_PC, DIM], dt.bfloat16)
    nc.vector.tensor_add(pvt, psv[0:POOL_PC, :], Bf["bv"][0:POOL_PC, :])
    nc.sync.dma_start(
        g["contrib"][0, OFF_PV:AG_TOT].rearrange("(n c) -> n c", n=POOL_PC), pvt)
    if "pk" in dbg:
        nc.sync.dma_start(dbg["pk"][:, :, :].rearrange("p a n -> (p a n)"),
                          g["contrib"][0, OFF_PK:OFF_PV])
        nc.sync.dma_start(dbg["pv"][:, :].rearrange("n c -> (n c)"),
                          g["contrib"][0, OFF_PV:AG_TOT])

    # ---- q^T for this core's 5 windows ----
    xtw = sb.tile([128, NC4, WPC * NQ], dt.bfloat16)
    nc.sync.dma_start_transpose(
        xtw, g["x_win"][:, :, :, :].rearrange("w t p c -> (w t p) c"))
    for oc in range(NC4):
        for piece in range(4):
            s = piece * 320
            ps = psA.tile([128, DIM], dt.float32, tag="a512")
            for ic in range(NC4):
                nc.tensor.matmul(ps[:, 0:320], W["wq"][:, ic, oc * 128:(oc + 1) * 128],
                                 xtw[:, ic, s:s + 320],
                                 start=(ic == 0), stop=(ic == NC4 - 1))
            nc.scalar.activation(qT[:, oc, s:s + 320], ps[:, 0:320], AF.Identity,
                                 bias=Bp["bq"][:, oc:oc + 1])
    if "q" in dbg:
        nc.sync.dma_start(dbg["q"][:, :, :], qT)

    # ---- per-window mask flags -> flb [128, 5] ----
    mwt = sb.tile([L_T, WPC * WN], dt.float32)
    nc.sync.dma_start(mwt, g["mask_win"][:, :, :].rearrange("l w p -> l (w p)"))
    mx = sb.tile([L_T, WPC, 1], dt.float32)
    nc.vector.reduce_max(mx, mwt.rearrange("l (w p) -> l w p", w=WPC),
                         axis=AX.X, opt_input=False, opt_output=False)
    psf = psA.tile([128, DIM], dt.float32, tag="a512")
    nc.tensor.matmul(psf[0:1, 0:WPC], ones2, mx[:, :, 0], start=True, stop=True)
    fl = sb.tile([1, WPC], dt.float32)
    nc.scalar.activation(fl, psf[0:1, 0:WPC], AF.Sign)
    psf2 = psA.tile([128, DIM], dt.float32, tag="a512")
    nc.tensor.matmul(psf2[:, 0:WPC], ones1, fl, start=True, stop=True)
    nc.vector.tensor_copy(flb, psf2[:, 0:WPC])
    if "flags" in dbg:
        nc.sync.dma_start(dbg["flags"][:, :], flb)


def _modulator(nc, tc, sb, psA, psA2, mw, mod_in, hs, tag):
    """Modulator on transposed f01 [128, 4, 960] bf16. Returns natural
    64-pitch pixel tile [128, 8, 4, 128] (pix = f*512+y*64+x, rows 1..9)."""
    outT = sb.tile([128, NC4, 2, STRIP_H, 64], dt.bfloat16, tag=f"modo{tag}",
                   name=f"modo{tag}")
    nc.vector.memset(outT[:, :, :, :, 48:64], 0.0)
    with tc.tile_pool(name=f"sbM{tag}", bufs=1) as sbm:
        # conv1 1x1 512->128 + bias + leaky relu -> padded [128, 2, 10, 50]
        lx1 = sbm.tile([128, 2, HP, 50], dt.bfloat16, tag="lx1")
        nc.vector.memset(lx1[:, :, :, 0:1], 0.0)
        nc.vector.memset(lx1[:, :, :, 49:50], 0.0)
        for f in range(2):
            ps = psA2.tile([128, HP * WI], dt.float32, tag="mid")
            for ic in range(NC4):
                nc.tensor.matmul(ps, mw["sq"][:, ic, :],
                                 mod_in[:, ic, f * 480:(f + 1) * 480],
                                 start=(ic == 0), stop=(ic == NC4 - 1))
            nc.scalar.activation(lx1[:, f, :, 1:49],
                                 ps.rearrange("p (y x) -> p y x", y=HP),
                                 AF.Lrelu, bias=mw["sqb"][:, 0:1], alpha=0.2)
        # zero the halo rows at image edges
        for f in range(2):
            nc.vector.tensor_scalar_mul(lx1[:, f, 0, :], lx1[:, f, 0, :],
                                        hs[:, 0:1])
            nc.vector.tensor_scalar_mul(lx1[:, f, HP - 1, :],
                                        lx1[:, f, HP - 1, :], hs[:, 1:2])
        # conv2 3x3 128->128 + bias + SiLU
        lx2 = sbm.tile([128, 2, STRIP_H * WI], dt.bfloat16, tag="lx2")
        for f in range(2):
            ps = psA2.tile([128, STRIP_H * WI], dt.float32, tag="mid")
            ti = 0
            for dy in (-1, 0, 1):
                for dx in (-1, 0, 1):
                    rhs = lx1[:, f, 1 + dy:1 + dy + STRIP_H, 1 + dx:1 + dx + WI]
                    nc.tensor.matmul(ps, mw["f"][:, ti, :], rhs,
                                     start=(ti == 0), stop=(ti == 8))
                    ti += 1
            nc.scalar.activation(lx2[:, f, :], ps, AF.Silu, bias=mw["fb"][:, 0:1])
        # conv3 1x1 128->512 + bias -> transposed 64-pitch tile
        for f in range(2):
            for oc in range(NC4):
                ps = psA2.tile([128, STRIP_H * WI], dt.float32, tag="mid")
                nc.tensor.matmul(ps, mw["un"][:, oc * 128:(oc + 1) * 128],
                                 lx2[:, f, :], start=True, stop=True)
                nc.scalar.activation(outT[:, oc, f, :, 0:48],
                                     ps.rearrange("p (y x) -> p y x", y=STRIP_H),
                                     AF.Identity, bias=mw["unb"][:, oc:oc + 1])
    # back to natural pixel-major via XBAR: [128pix, 8 chunks, 4oc, 128]
    nat = sb.tile([128, 8, NC4, 128], dt.bfloat16, tag=f"nat{tag}",
                  name=f"nat{tag}")
    for oc in range(NC4):
        nc.sync.dma_start_transpose(
            nat[:, :, oc, :],
            outT[:, oc, :, :, :].rearrange("p f y x -> p (f y x)"))
    return nat


def _stage_b(nc, tc, g, sb, sb1, W, Bf, qT, flb, ones_b, dbg):
    ag = g["agout"]
    kgv = ag[:, OFF_K:OFF_V].rearrange("s (y x t c) -> s y x t c",
                                       y=STRIP_H, x=64, t=T)
    vgv = ag[:, OFF_V:OFF_PK].rearrange("s (y x t c) -> s y x t c",
                                        y=STRIP_H, x=64, t=T)

    # pooled keys resident
    pk = sb1.tile([128, NC4, N_CORES * POOL_PC], dt.bfloat16)
    for s in range(N_CORES):
        nc.sync.dma_start(
            pk[:, :, s * POOL_PC:(s + 1) * POOL_PC],
            ag[s, OFF_PK:OFF_PV].rearrange("(p a n) -> p a n", p=128, a=NC4))
    pv = sb1.tile([128, NPCH, DIM], dt.bfloat16)
    row = 0
    for s in range(N_CORES):
        r = 0
        while r < POOL_PC:
            ch, off = (row + r) // 128, (row + r) % 128
            n = min(POOL_PC - r, 128 - off)
            nc.sync.dma_start(
                pv[off:off + n, ch, :],
                ag[s, OFF_PV + r * DIM:OFF_PV + (r + n) * DIM]
                .rearrange("(n c) -> n c", c=DIM))
            r += n
        row += POOL_PC

    # dynamic-offset registers
    meta_t = sb1.tile([1, META_LEN], dt.int32)
    nc.sync.dma_start(meta_t, g["meta"][:].unsqueeze(0))
    regs = []
    for i in range(WPC * REGS_PER_WIN):
        r = nc.alloc_register(mybir.EngineType.SP, f"mreg{i}")
        nc.sync.reg_load(r, meta_t[0:1, i:i + 1])
        regs.append(nc.sync.snap(r))

    # ---- pool-key scores + exp, batched over all 5 windows ----
    pTpool = sb1.tile([128, N_HEAD, NPCH, WPC * NQ], dt.bfloat16)
    with tc.tile_pool(name="psQ", bufs=2, space="PSUM") as psQ:
        for h in range(N_HEAD):
            for c in range(NPCH):
                n = min(128, NPOOL - c * 128)
                ps = psQ.tile([128, WPC * NQ], dt.float32, tag="q")
                for p0 in range(0, WPC * NQ, DIM):
                    p1 = min(WPC * NQ, p0 + DIM)
                    nc.tensor.matmul(ps[0:n, p0:p1],
                                     pk[:, h, c * 128:c * 128 + n],
                                     qT[:, h, p0:p1], start=True, stop=True)
                nc.scalar.activation(pTpool[0:n, h, c, :], ps[0:n, :],
                                     AF.Exp, scale=SCALE)

    with (
        tc.tile_pool(name="ps_s", bufs=2, space="PSUM") as ps_s,
        tc.tile_pool(name="ps_y", bufs=2, space="PSUM") as ps_y,
        tc.tile_pool(name="ps_d", bufs=1, space="PSUM") as ps_d,
        tc.tile_pool(name="ps_p", bufs=1, space="PSUM") as ps_p,
    ):
        _windows(nc, tc, g, sb, ps_s, ps_y, ps_d, ps_p, W, Bf, qT, flb,
                 ones_b, dbg, regs, kgv, vgv, pv, pTpool)


def _windows(nc, tc, g, sb, ps_s, ps_y, ps_d, ps_p, W, Bf, qT, flb,
             ones_b, dbg, regs, kgv, vgv, pv, pTpool):
    ds = bass.ds
    for wi in range(WPC):
        r_prev, r_cur, r_next, r_x0, r_xo, r_x12 = \
            regs[wi * REGS_PER_WIN:(wi + 1) * REGS_PER_WIN]

        # ---- patch gathers (natural layout, own-pixels-first) ----
        kp = sb.tile([128, 8, DIM], dt.bfloat16, tag="kp")
        vp = sb.tile([128, 8, DIM], dt.bfloat16, tag="vp")
        for dst, src in ((kp, kgv), (vp, vgv)):
            # own window: chunk 2f partitions 0:64
            nc.sync.dma_start(
                dst[0:64, 0:8:2, :],
                src[ds(r_cur, 1), :, ds(r_xo, 8), :, :].squeeze(0))
            # top margin rows (prev strip rows 4:8): chunk 2f partitions 64:128
            nc.sync.dma_start(
                dst[64:128, 0:8:2, :],
                src[ds(r_prev, 1), 4:8, ds(r_x0, 16), :, :].squeeze(0))
            # bottom margin rows (next strip rows 0:4): chunk 2f+1 p 0:64
            nc.sync.dma_start(
                dst[0:64, 1:8:2, :],
                src[ds(r_next, 1), 0:4, ds(r_x0, 16), :, :].squeeze(0))
            # left margin cols: chunk 2f+1 p 64:96
            nc.sync.dma_start(
                dst[64:96, 1:8:2, :],
                src[ds(r_cur, 1), :, ds(r_x0, 4), :, :].squeeze(0))
            # right margin cols: chunk 2f+1 p 96:128
            nc.sync.dma_start(
                dst[96:128, 1:8:2, :],
                src[ds(r_cur, 1), :, ds(r_x12, 4), :, :].squeeze(0))
        # k patch transpose: [128ch, (slot, cchunk), 128pix]
        kpT = sb.tile([128, 32, 128], dt.bfloat16, tag="kpT")
        nc.sync.dma_start_transpose(
            kpT, kp[:, :, :].rearrange("p s c -> p (s c)"))
        if wi == 0 and "kp" in dbg:
            nc.sync.dma_start(dbg["kp"][:, :, :], kp)
            nc.sync.dma_start(dbg["vp"][:, :, :], vp)
            nc.sync.dma_start(dbg["kpT"][:, :, :], kpT)

        q_w = qT[:, :, wi * NQ:(wi + 1) * NQ]  # [128, 4h, 256]
        yfin = [sb.tile([128, DIM], dt.bfloat16, tag=f"yfin{h2}", name=f"yfin{h2}")
                for h2 in range(2)]

        for h in range(N_HEAD):
            # ---- patch scores + exp (2 groups of 4 chunks) ----
            pT = sb.tile([128, NPATCH, NQ], dt.bfloat16, tag="pT", name="pT")
            for g0 in (0, 4):
                ps = ps_s.tile([128, 4, NQ], dt.float32, tag="s")
                for j in range(g0, g0 + 4):
                    nc.tensor.matmul(ps[:, j - g0, :], kpT[:, j * 4 + h, :],
                                     q_w[:, h, :], start=True, stop=True)
                nc.scalar.activation(pT[:, g0:g0 + 4, :], ps, AF.Exp,
                                     scale=SCALE)
            if wi == 0 and h == 0 and "pT" in dbg:
                nc.sync.dma_start(dbg["pT"][:, 0:NPATCH, :], pT)
                nc.sync.dma_start(
                    dbg["pT"][:, NPATCH:NCHUNK, :],
                    pTpool[:, 0, :, 0:NQ])

            # ---- denominators (tiny ap=1 matmuls) ----
            dn = ps_d.tile([128, 8], dt.float32, tag="d")
            for half in range(2):
                q0 = wi * NQ + half * 128
                for j in range(NPATCH):
                    nc.tensor.matmul(dn[:, half:half + 1],
                                     pT[:, j, half * 128:(half + 1) * 128],
                                     ones_b,
                                     start=(j == 0), stop=False)
                for c in range(NPCH):
                    n = min(128, NPOOL - c * 128)
                    nc.tensor.matmul(dn[:, half:half + 1],
                                     pTpool[0:n, h, c, q0:q0 + 128],
                                     ones_b[0:n, :],
                                     start=False, stop=(c == NPCH - 1))
            for f in range(T):
                nc.tensor.matmul(
                    dn[(f % 2) * 64:(f % 2) * 64 + 64, 2 + f // 2:3 + f // 2],
                    pT[0:64, 2 * f, f * WN:(f + 1) * WN], ones_b[0:64, :],
                    start=True, stop=True)

            # ---- AV (transposed): yT [q, ch] ----
            ys = ps_y.tile([128, 4, 128], dt.float32, tag="y")
            for half in range(2):
                q0 = wi * NQ + half * 128
                for j in range(NPATCH):
                    nc.tensor.matmul(ys[:, half, :],
                                     pT[:, j, half * 128:(half + 1) * 128],
                                     vp[:, j, h * 128:(h + 1) * 128],
                                     start=(j == 0), stop=False)
                for c in range(NPCH):
                    n = min(128, NPOOL - c * 128)
                    nc.tensor.matmul(ys[:, half, :],
                                     pTpool[0:n, h, c, q0:q0 + 128],
                                     pv[0:n, c, h * 128:(h + 1) * 128],
                                     start=False, stop=(c == NPCH - 1))
            for f in range(T):
                nc.tensor.matmul(
                    ys[(f % 2) * 64:(f % 2) * 64 + 64, 2 + f // 2, :],
                    pT[0:64, 2 * f, f * WN:(f + 1) * WN],
                    vp[0:64, 2 * f, h * 128:(h + 1) * 128],
                    start=True, stop=True)

            # ---- normalize + blend ----
            rec = sb.tile([128, 4], dt.float32, tag="rec", name="rec")
            nc.vector.reciprocal(rec, dn[:, 0:4])
            for half in range(2):
                yln = sb.tile([128, 128], dt.bfloat16, tag="yln", name="yln")
                nc.vector.tensor_scalar_mul(yln, ys[:, 2 + half, :],
                                            rec[:, 2 + half:3 + half])
                dlt = sb.tile([128, 128], dt.bfloat16, tag="dlt", name="dlt")
                nc.vector.scalar_tensor_tensor(
                    dlt, ys[:, half, :], rec[:, half:half + 1], yln,
                    ALU.mult, ALU.subtract)
                nc.vector.scalar_tensor_tensor(
                    yfin[half][:, h * 128:(h + 1) * 128], dlt,
                    flb[:, wi:wi + 1], yln, ALU.mult, ALU.add)

        # ---- output projection ----
        for half in range(2):
            yTT = sb.tile([128, NC4, 128], dt.bfloat16, tag="yTT", name="yTT")
            nc.sync.dma_start_transpose(yTT, yfin[half][:, :])
            pso = ps_p.tile([128, DIM], dt.float32, tag="proj")
            for h in range(N_HEAD):
                nc.tensor.matmul(pso, yTT[:, h, :], W["wp"][:, h, :],
                                 start=(h == 0), stop=(h == N_HEAD - 1))
            osb = sb.tile([128, DIM], dt.float32, tag="osb", name="osb")
            nc.vector.tensor_add(osb, pso, Bf["bp"])
            nc.sync.dma_start(
                g["out_win"][wi, 2 * half:2 * half + 2, :, :]
                .rearrange("t p c -> (t p) c"), osb)


# ==================== host side ====================

def _host_inputs(inputs, debug=False):
    x = np.asarray(inputs["x"], np.float32)[0]  # [4,48,48,512]
    mask = np.asarray(inputs["mask"], np.float32)[0, :, :, :, 0]  # [2,48,48]
    xb = x.astype(BF)
    maskb = mask.astype(BF)

    pool_ind = np.zeros((128, 9, POOL_PC), BF)
    for n in range(9):
        for p in range(128):
            pix = n * 128 + p
            y, xx = pix // WI, pix % WI
            pool_ind[p, n, (y // PH) * PGRID + xx // PW] = BF(1.0 / (PH * PW))

    common = dict(
        wq=np.asarray(inputs["Wq"], np.float32).astype(BF),
        wk=np.asarray(inputs["Wk"], np.float32).astype(BF),
        wv=np.asarray(inputs["Wv"], np.float32).astype(BF),
        wp=np.asarray(inputs["Wp"], np.float32).astype(BF),
        bq=np.asarray(inputs["bq"], np.float32),
        bk=np.asarray(inputs["bk"], np.float32),
        bv=np.asarray(inputs["bv"], np.float32),
        bp=np.asarray(inputs["bp"], np.float32),
        pool_b=np.asarray(inputs["pool_b"], np.float32),
        pool_ind=pool_ind,
    )
    for tag, pre in (("k", "kmod"), ("v", "vmod")):
        common[f"{tag}sq"] = np.ascontiguousarray(
            np.asarray(inputs[f"{pre}_sq_w"], np.float32)[:, :, 0, 0].T).astype(BF)
        common[f"{tag}sqb"] = np.asarray(inputs[f"{pre}_sq_b"], np.float32)
        fw = np.asarray(inputs[f"{pre}_f_w"], np.float32)
        common[f"{tag}f"] = np.ascontiguousarray(
            np.stack([fw[:, :, dy, dx].T for dy in range(3) for dx in range(3)])
        ).astype(BF)
        common[f"{tag}fb"] = np.asarray(inputs[f"{pre}_f_b"], np.float32)
        common[f"{tag}un"] = np.ascontiguousarray(
            np.asarray(inputs[f"{pre}_un_w"], np.float32)[:, :, 0, 0].T).astype(BF)
        common[f"{tag}unb"] = np.asarray(inputs[f"{pre}_un_b"], np.float32)

    in_maps = []
    for c in range(N_CORES):
        m = dict(common)
        if c < N_STRIP:
            r0 = c * STRIP_H
            xs = np.zeros((T, HP, WI, DIM), BF)
            ms = np.zeros((L_T, HP, WI), BF)
            lo, hi = max(0, r0 - 1), min(HI, r0 + STRIP_H + 1)
            xs[:, lo - (r0 - 1):lo - (r0 - 1) + hi - lo] = xb[:, lo:hi]
            ms[:, lo - (r0 - 1):lo - (r0 - 1) + hi - lo] = maskb[:, lo:hi]
            m["x_strip"] = xs
            m["mask_strip"] = ms
            m["halo_scale"] = np.array(
                [0.0 if r0 == 0 else 1.0,
                 0.0 if r0 + STRIP_H == HI else 1.0], np.float32)
        else:
            m["x_strip"] = np.zeros((T, HP, WI, DIM), BF)
            m["mask_strip"] = np.zeros((L_T, HP, WI), BF)
            m["halo_scale"] = np.ones(2, np.float32)
        xw = np.zeros((WPC, T, WN, DIM), BF)
        mw = np.zeros((L_T, WPC, WN), np.float32)
        for k, w in enumerate(WIN_ASSIGN[c]):
            i, j = w // N_WW, w % N_WW
            blk = xb[:, 8 * i:8 * i + 8, 8 * j:8 * j + 8, :]
            xw[k] = blk.reshape(T, WN, DIM)
            mw[:, k] = mask[:, 8 * i:8 * i + 8, 8 * j:8 * j + 8].reshape(L_T, WN)
        m["x_win"] = xw
        m["mask_win"] = mw
        f, pr0 = c // 2, 6 * (c % 2)
        m["x_pool"] = np.ascontiguousarray(xb[f, pr0 * 4:pr0 * 4 + 24])
        m["meta"] = _meta_for_core(c)
        in_maps.append(m)
    return in_maps


def _get_nc(debug=False):
    key = bool(debug)
    if key not in _NC_CACHE:
        _NC_CACHE[key] = build_nc(debug=debug)
    return _NC_CACHE[key]


def run_spmd(inputs, debug=False):
    nc = _get_nc(debug=debug)
    in_maps = _host_inputs(inputs, debug=debug)
    res = run_bass_kernel_spmd(nc, in_maps, list(range(N_CORES)))
    return res


def assemble(results):
    out = np.zeros((T, HI, WI, DIM), np.float32)
    done = set()
    for c in range(N_CORES):
        ow = results[c]["out_win"]
        for k, w in enumerate(WIN_ASSIGN[c]):
            if w in done:
                continue
            done.add(w)
            i, j = w // N_WW, w % N_WW
            out[:, 8 * i:8 * i + 8, 8 * j:8 * j + 8, :] = \
                ow[k].reshape(T, WH, WW, DIM)
    return out[None]


def kernel(**inputs):
    res = run_spmd(inputs)
    return assemble(res.results)
